# revision 17
# baseline (speedup 1.0000x reference)
"""Trainium2 Bass kernel for nn_PointEncoder (B=16, N=8192, L=512, D=384).

Sharding: data-parallel over batch, 2 batches per NeuronCore x 8 cores,
no collectives; full inputs sharded / outputs gathered on host.

v2 design (fp8e4 DoubleRow matmuls at 2x PE throughput):
  * MLP layers 1-3, ctx stats, scores, V, attn@V, the softmax
    denominator and the whole GEGLU FF all run as fp8 DoubleRow
    matmuls (0.5 cyc/row).  K dims are padded to 4x128; the pad
    subtile of each h tile carries a ones-row so every layer bias
    rides its matmul for free (works for arbitrary biases).
  * LayerNorm mean subtraction is folded into column-centered weights
    on the host (exact identity).  Only sum / sum-of-squares stats are
    computed on chip, directly in per-point column form via tiny
    DoubleRow matmuls; 1/sigma is applied through the ACT scale
    operand of the softmax exp and an ACT Copy-scale on V.
  * W3 (the linear last MLP layer) is folded into the score/V weights;
    ctx itself is only materialized as its square (for the variance)
    straight out of PSUM.
  * exp uses a constant -1 logit shift (cancels in softmax exactly) to
    bound fp8 magnitudes; logits are provably small, no max needed.
  * All data-dependent scale-compensation constants stream in as tiny
    column tensors, so the program is input-independent and compiles
    exactly once.

Engine split per 512-point chunk: PE ~20 DoubleRow + 5 f32r matmuls;
DVE: relu(h0,h1,h2) out of PSUM; ACT: square(ctx), exp x4, V-scale x4;
Pool (GpSimd, SBUF-only): tiny column math.  The per-batch epilogue
(attn normalize, output projections, FF stats from x1n rows, fp8
GEGLU) is software-pipelined across the next batch's chunk stream.
"""

import math
import numpy as np
import ml_dtypes

import concourse.bass as bass
import concourse.tile as tile
import concourse.mybir as mybir
from concourse import bacc

P = 128
B, N_FULL, L, D = 16, 8192, 512, 384
FF = 4 * D  # 1536
FF2 = 2 * FF  # 3072
DT = D // P  # 3
KT = 4      # padded K subtiles for D-contractions
LT = L // P  # 4
FFT = FF // P  # 12
CHUNK = 512
CT = CHUNK // P  # 4
NCORES = 8
BPC = B // NCORES  # 2

f32 = mybir.dt.float32
f32r = mybir.dt.float32r
bf16 = mybir.dt.bfloat16
fp8 = mybir.dt.float8e4
AF = mybir.ActivationFunctionType
ALU = mybir.AluOpType
DR = mybir.MatmulPerfMode.DoubleRow

EPS = 1e-5
SCALE = 1.0 / math.sqrt(D)
ESHIFT = -1.0  # constant logit shift inside exp; cancels in softmax

_tables_patched = False


def _patch_act_tables():
    """Steer the table chooser to 'natural_log_exp_and_others' (ln, exp,
    relu, square, copy) so the chunk stream needs no table swaps; only the
    epilogue Gelu block loads its own set."""
    global _tables_patched
    if _tables_patched:
        return
    from concourse import hw_specs, bacc as _bacc
    orig = hw_specs.get_activation_tables

    def patched(arch):
        t = dict(orig(arch))
        if "natural_log_exp_and_others" in t:
            if "exp_and_others" in t:
                t["exp_and_others"] = t["exp_and_others"] - {AF.Exp}
            if "natural_log" in t:
                t["natural_log"] = t["natural_log"] - {AF.Ln}
        return t

    _bacc.get_activation_tables = patched
    _tables_patched = True


def build_nc(n_points=N_FULL, bpc=BPC, gelu_af=None):
    import os
    STOP = int(os.environ.get("KSTOP", "5"))
    SUB = int(os.environ.get("KSUB", "9"))
    if gelu_af is None:
        gelu_af = AF.Gelu
    nchunks = n_points // CHUNK
    _patch_act_tables()
    nc = bacc.Bacc("TRN2", target_bir_lowering=False, debug=False,
                   enable_asserts=False)

    def di(name, shape, dtype=f32):
        return nc.dram_tensor(name, list(shape), dtype,
                              kind="ExternalInput").ap()

    xT = di("xT", [bpc, 4, n_points], f32r)     # row 3 = ones
    w0 = di("w0", [4, D], f32r)                 # [W0; b0] * s0
    w1 = di("w1", [P * KT, D], fp8)             # bias row at K=384, pad 0
    w2 = di("w2", [P * KT, D], fp8)
    w3 = di("w3", [P * KT, D], fp8)             # ctx (squared) path
    w3q = di("w3q", [P * KT, L], fp8)           # W3 @ centered wq2
    w3v = di("w3v", [P * KT, D], fp8)           # W3 @ centered gwv
    w3s = di("w3s", [P * KT, 1], fp8)           # ctx row-sum weights
    wo = di("wo", [D, D], f32r)                 # wo / k8v
    fw1 = di("fw1", [P * KT, FF2], fp8)         # centered, gain-folded
    fw2 = di("fw2", [FF, D], fp8)
    lqbT = di("lqbT", [D, L])                   # lq.T + bo'
    lqbn = di("lqbn", [L, D])                   # lq + bo' + fb2
    ident_d = di("ident", [P, P])
    onesr_d = di("onesr", [1, P])
    ones16_d = di("ones16", [P * KT, 16], fp8)  # col 0 = 1 (16-wide for DR)
    padh_d = di("padh", [P, CHUNK], fp8)        # row 0 = 1, rest 0
    padz_d = di("padz", [P, CHUNK], fp8)        # zeros
    # data-dependent per-partition constants (columns)
    cols_d = {n: di(n, [P, 1]) for n in
              ["ch1", "ch2", "csq", "cc1", "clnv", "cbe", "ccvr",
               "ccgs", "ccy", "ccb3"]}
    bua = di("bua", [P, FFT])    # kfa * bu_a, [p, mt]
    bug = di("bug", [P, FFT])    # bu_g, [p, mt]
    y = nc.dram_tensor("y", [bpc, L, D], f32, kind="ExternalOutput").ap()

    with tile.TileContext(nc) as tc:
        with tc.tile_pool(name="singles", bufs=1) as singles, \
             tc.tile_pool(name="work", bufs=1) as work, \
             tc.tile_pool(name="psum", bufs=1, space="PSUM") as psum:

            # ---------------- load params ----------------
            def ld(name, shape, dtype, src, eng=None):
                t = singles.tile(shape, dtype, name=name)
                (eng or nc.sync).dma_start(t, src)
                return t

            r4 = lambda a: a.rearrange("(t p) m -> p t m", p=P)

            xT_pre = work.tile([4, CHUNK], f32r, tag="xT", bufs=2,
                               name="xT_pre")
            nc.sync.dma_start(xT_pre, xT[0, :, 0:CHUNK])
            w0_sb = ld("w0_sb", [4, D], f32r, w0)
            w1_sb = ld("w1_sb", [P, KT, D], fp8, r4(w1))
            w2_sb = ld("w2_sb", [P, KT, D], fp8, r4(w2))
            w3_sb = ld("w3_sb", [P, KT, D], fp8, r4(w3))
            w3q_sb = ld("w3q_sb", [P, KT, L], fp8, r4(w3q))
            w3v_sb = ld("w3v_sb", [P, KT, D], fp8, r4(w3v))
            w3s_sb = ld("w3s_sb", [P, KT, 1], fp8, r4(w3s))
            colc = {n: ld(n + "_sb", [P, 1], f32, cols_d[n])
                    for n in cols_d}
            g = nc.gpsimd
            wo_sb = ld("wo_sb", [P, DT, D], f32r, r4(wo), eng=g)
            fw1_sb = ld("fw1_sb", [P, KT, FF2], fp8, r4(fw1), eng=g)
            fw2_sb = ld("fw2_sb", [P, FFT, D], fp8, r4(fw2), eng=g)
            lqbT_sb = ld("lqbT_sb", [P, DT, L], f32, r4(lqbT), eng=g)
            lqbn_sb = ld("lqbn_sb", [P, LT, D], f32,
                         lqbn.rearrange("(t p) d -> p t d", p=P), eng=g)
            bua_sb = ld("bua_sb", [P, FFT], f32, bua, eng=g)
            bug_sb = ld("bug_sb", [P, FFT], f32, bug, eng=g)
            ident = ld("ident_sb", [P, P], f32r, ident_d, eng=g)
            ones_row = ld("ones_row", [1, P], f32r, onesr_d, eng=g)
            ones16 = ld("ones16", [P, KT, 16], fp8, r4(ones16_d), eng=g)

            eps_c = singles.tile([P, 1], f32, name="eps_c")
            nc.vector.memset(eps_c, EPS)
            neg1_c = singles.tile([P, 1], f32, name="neg1_c")
            nc.vector.memset(neg1_c, ESHIFT)

            # persistent double-buffered h tiles; pad subtile 3 is zero with
            # a ones-row at partition 0 (K row 384) to carry biases.
            def padded_pair(name, pad_src):
                ts = []
                for i in range(2):
                    t = singles.tile([P, KT, CHUNK], fp8, name=f"{name}{i}")
                    nc.gpsimd.dma_start(t[:, 3, :], pad_src)
                    ts.append(t)
                return ts

            h0b = padded_pair("h0", padh_d)
            h1b = padded_pair("h1", padh_d)
            h2b = padded_pair("h2", padh_d)
            sqb = padded_pair("sq", padz_d)

            fT = singles.tile([P, KT, L], fp8, name="fT")
            nc.gpsimd.dma_start(fT[:, 3, :], padz_d)

            PAIRS = ((0, 2), (2, 4))

            def _run():
                pending = None       # (epi2_closure, epi3_closure)
                pending_res = None

                def flush_pending():
                    nonlocal pending, pending_res
                    if pending is not None:
                        if pending_res is None:
                            pending_res = pending[0]()
                        pending[1](pending_res)
                        pending = None
                        pending_res = None

                for b in range(bpc):
                    # 3 banks attn accumulators + bank 3 = den row
                    acc = psum.tile([P, KT, L], f32, tag="acc", name=f"acc{b}")

                    def stage_a(c, b=b, acc=acc):
                        uid = f"{b}_{c}"
                        hi = c % 2
                        if b == 0 and c == 0:
                            xT_c = xT_pre
                        else:
                            xT_c = work.tile([4, CHUNK], f32r, tag="xT",
                                             bufs=2, name=f"xT{uid}")
                            nc.sync.dma_start(
                                xT_c, xT[b, :, c * CHUNK:(c + 1) * CHUNK])
                        h0, h1, h2, sq = h0b[hi], h1b[hi], h2b[hi], sqb[hi]
                        # L0 (f32r, K=4): relu on DVE
                        for mt in range(DT):
                            ps = psum.tile([P, CHUNK], f32, tag="work",
                                           bufs=3, name=f"ps0{mt}_{uid}")
                            nc.tensor.matmul(ps, w0_sb[:, mt * P:(mt + 1) * P],
                                             xT_c, start=True, stop=True)
                            nc.vector.tensor_scalar(
                                out=h0[:, mt, :], in0=ps, scalar1=0.0,
                                scalar2=None, op0=ALU.max)

                        def mm_dr(ps_out, w_sb, mt, rhs):
                            for ks in PAIRS:
                                nc.tensor.matmul(
                                    ps_out,
                                    w_sb[:, ks[0]:ks[1], mt * P:(mt + 1) * P],
                                    rhs[:, ks[0]:ks[1], :],
                                    start=(ks[0] == 0), stop=(ks[0] == 2),
                                    perf_mode=DR)

                        # L1, L2: relu via DVE TSP (mult comp-scale, max 0)
                        for li, (w_sb, hin, hout, cname) in enumerate(
                                ((w1_sb, h0, h1, "ch1"),
                                 (w2_sb, h1, h2, "ch2"))):
                            for mt in range(DT):
                                ps = psum.tile([P, CHUNK], f32, tag="work",
                                               bufs=3,
                                               name=f"ps{li + 1}{mt}_{uid}")
                                mm_dr(ps, w_sb, mt, hin)
                                nc.vector.tensor_scalar(
                                    out=hout[:, mt, :], in0=ps,
                                    scalar1=colc[cname], scalar2=0.0,
                                    op0=ALU.mult, op1=ALU.max)
                        # L3 -> ctx materialized only as its square (ACT)
                        for mt in range(DT):
                            ps = psum.tile([P, CHUNK], f32, tag="work",
                                           bufs=3, name=f"ps3{mt}_{uid}")
                            mm_dr(ps, w3_sb, mt, h2)
                            nc.scalar.activation(sq[:, mt, :], ps, AF.Square,
                                                 scale=colc["csq"])

                        # per-point stats, column form: S1 ~ sum(ctx),
                        # S2 ~ sum(ctx^2); tiny DoubleRow matmuls
                        st_ps = psum.tile([P, 2, CT], f32, tag="stat",
                                          bufs=1, name=f"st{uid}")
                        for jt in range(CT):
                            for kt in range(DT):
                                nc.tensor.matmul(
                                    st_ps[:, 0, jt:jt + 1],
                                    h2[:, kt, jt * P:(jt + 1) * P],
                                    w3s_sb[:, kt, :],
                                    start=(kt == 0), stop=(kt == DT - 1),
                                    skip_group_check=True)
                                nc.tensor.matmul(
                                    st_ps[:, 1, jt:jt + 1],
                                    sq[:, kt, jt * P:(jt + 1) * P],
                                    ones16[:, kt, 0:1],
                                    start=(kt == 0), stop=(kt == DT - 1),
                                    skip_group_check=True)
                        # column math -> a_e (exp scale), a_v (V scale)
                        s1c = work.tile([P, CT], f32, tag="col", bufs=4,
                                        name=f"s1c{uid}")
                        nc.vector.tensor_scalar(
                            out=s1c, in0=st_ps[:, 0, :],
                            scalar1=colc["ccb3"], scalar2=None, op0=ALU.add)
                        sq1 = work.tile([P, CT], f32, tag="col", bufs=4,
                                        name=f"sq1{uid}")
                        nc.vector.tensor_tensor(sq1, s1c, s1c, ALU.mult)
                        U = work.tile([P, CT], f32, tag="col", bufs=4,
                                      name=f"U{uid}")
                        nc.vector.scalar_tensor_tensor(
                            U, st_ps[:, 1, :], colc["cc1"], sq1,
                            ALU.mult, ALU.subtract)
                        lnv = work.tile([P, CT], f32, tag="col", bufs=4,
                                        name=f"lnv{uid}")
                        nc.scalar.activation(lnv, U, AF.Ln, bias=eps_c,
                                             scale=colc["clnv"])
                        a_e = work.tile([P, CT], f32, tag="acol", bufs=2,
                                        name=f"ae{uid}")
                        nc.scalar.activation(a_e, lnv, AF.Exp,
                                             bias=colc["cbe"], scale=-0.5)
                        a_v = work.tile([P, CT], f32, tag="acol", bufs=2,
                                        name=f"av{uid}")
                        nc.vector.tensor_scalar(
                            out=a_v, in0=a_e, scalar1=colc["ccvr"],
                            scalar2=None, op0=ALU.mult)
                        return hi, a_e, a_v

                    def stage_b(c, hi, a_e, a_v, b=b, acc=acc):
                        if STOP < 2:
                            return
                        uid = f"{b}_{c}"
                        h2 = h2b[hi]
                        v_t = work.tile([P, CT, D], fp8, tag="v", bufs=2,
                                        name=f"v{uid}")
                        e_t = work.tile([P, CT, L], fp8, tag="e", bufs=2,
                                        name=f"e{uid}")
                        for jt in range(CT):
                            psv = psum.tile([P, D], f32, tag="work", bufs=3,
                                            name=f"psv{jt}_{uid}")
                            for ks in PAIRS:
                                nc.tensor.matmul(
                                    psv,
                                    h2[:, ks[0]:ks[1], jt * P:(jt + 1) * P],
                                    w3v_sb[:, ks[0]:ks[1], :],
                                    start=(ks[0] == 0), stop=(ks[0] == 2),
                                    perf_mode=DR)
                            nc.scalar.activation(v_t[:, jt, :], psv, AF.Copy,
                                                 scale=a_v[:, jt:jt + 1])
                            pss = psum.tile([P, L], f32, tag="work", bufs=3,
                                            name=f"pss{jt}_{uid}")
                            for ks in PAIRS:
                                nc.tensor.matmul(
                                    pss,
                                    h2[:, ks[0]:ks[1], jt * P:(jt + 1) * P],
                                    w3q_sb[:, ks[0]:ks[1], :],
                                    start=(ks[0] == 0), stop=(ks[0] == 2),
                                    perf_mode=DR)
                            nc.scalar.activation(e_t[:, jt, :], pss, AF.Exp,
                                                 bias=neg1_c,
                                                 scale=a_e[:, jt:jt + 1])
                        first, last = (c == 0), (c == nchunks - 1)
                        for pi, ks in enumerate(PAIRS):
                            for mt in range(DT):
                                nc.tensor.matmul(
                                    acc[:, mt, :],
                                    v_t[:, ks[0]:ks[1], mt * P:(mt + 1) * P],
                                    e_t[:, ks[0]:ks[1], :],
                                    start=(first and pi == 0),
                                    stop=(last and pi == 1),
                                    perf_mode=DR, skip_group_check=True)
                            for kd in (ks[0], ks[0] + 1):
                                nc.tensor.matmul(
                                    acc[0:1, 3, :],
                                    ones16[:, kd, 0:1],
                                    e_t[:, kd, :],
                                    start=(first and kd == 0),
                                    stop=(last and kd == 3),
                                    skip_group_check=True)

                    def epi1(b=b, acc=acc):
                        ub = f"b{b}"
                        if STOP < 3:
                            outn = work.tile([P, DT, L], f32r, tag="outn",
                                             bufs=1, name=f"outn{ub}")
                            nc.vector.memset(outn, 0.01)
                            return outn
                        rec = work.tile([1, L], f32r, tag="row", bufs=2,
                                        name=f"rec{ub}")
                        with nc.allow_low_precision("f32r is full fp32"):
                            nc.vector.reciprocal(rec, acc[0:1, 3, :])
                        ps_rb = psum.tile([P, L], f32, tag="work", bufs=3,
                                          name=f"psrb{ub}")
                        nc.tensor.matmul(ps_rb, ones_row, rec,
                                         start=True, stop=True)
                        rb = work.tile([P, L], f32, tag="rb", bufs=1,
                                       name=f"rb{ub}")
                        nc.vector.tensor_copy(rb, ps_rb)
                        outn = work.tile([P, DT, L], f32r, tag="outn", bufs=1,
                                         name=f"outn{ub}")
                        for mt in range(DT):
                            nc.vector.tensor_tensor(outn[:, mt, :],
                                                    acc[:, mt, :], rb,
                                                    ALU.mult)
                        return outn

                    def epi2(outn, b=b):
                        ub = f"b{b}"
                        if STOP < 4:
                            x1n = work.tile([P, LT, D], f32, tag="x1n",
                                            bufs=1, name=f"x1n{ub}")
                            nc.vector.memset(x1n, 0.01)
                            return x1n
                        x1T = work.tile([P, DT, L], f32, tag="x1T", bufs=1,
                                        name=f"x1T{ub}")
                        for mt in range(DT):
                            ps = psum.tile([P, L], f32, tag="work", bufs=3,
                                           name=f"px1T{mt}{ub}")
                            for kt in range(DT):
                                nc.tensor.matmul(
                                    ps, wo_sb[:, kt, mt * P:(mt + 1) * P],
                                    outn[:, kt, :],
                                    start=(kt == 0), stop=(kt == DT - 1))
                            nc.vector.tensor_tensor(x1T[:, mt, :], ps,
                                                    lqbT_sb[:, mt, :], ALU.add)
                        x1n = work.tile([P, LT, D], f32, tag="x1n", bufs=1,
                                        name=f"x1n{ub}")
                        if SUB < 2:
                            nc.vector.memset(x1n, 0.01)
                            nc.vector.memset(fT[:, 0:3, :], 0.01)
                            return x1n
                        for lt in range(LT):
                            ps = psum.tile([P, D], f32, tag="work", bufs=3,
                                           name=f"px1n{lt}{ub}")
                            for kt in range(DT):
                                nc.tensor.matmul(
                                    ps, outn[:, kt, lt * P:(lt + 1) * P],
                                    wo_sb[:, kt, :],
                                    start=(kt == 0), stop=(kt == DT - 1))
                            nc.vector.tensor_tensor(x1n[:, lt, :], ps,
                                                    lqbn_sb[:, lt, :], ALU.add)
                        if SUB < 3:
                            nc.vector.memset(fT[:, 0:3, :], 0.01)
                            return x1n
                        # FF LN stats, per-latent column form from x1n
                        sf1 = work.tile([P, LT], f32, tag="col", bufs=4,
                                        name=f"sf1{ub}")
                        sf2 = work.tile([P, LT], f32, tag="col", bufs=4,
                                        name=f"sf2{ub}")
                        x1sq = work.tile([P, D], f32, tag="x1sq", bufs=2,
                                         name=f"x1sq{ub}")
                        for lt in range(LT):
                            nc.vector.tensor_reduce(
                                sf1[:, lt:lt + 1], x1n[:, lt, :],
                                mybir.AxisListType.X, ALU.add)
                            nc.vector.tensor_tensor(x1sq, x1n[:, lt, :],
                                                    x1n[:, lt, :], ALU.mult)
                            nc.vector.tensor_reduce(
                                sf2[:, lt:lt + 1], x1sq,
                                mybir.AxisListType.X, ALU.add)
                        sqf = work.tile([P, LT], f32, tag="col", bufs=4,
                                        name=f"sqf{ub}")
                        nc.vector.tensor_tensor(sqf, sf1, sf1, ALU.mult)
                        Uf = work.tile([P, LT], f32, tag="col", bufs=4,
                                       name=f"Uf{ub}")
                        nc.vector.scalar_tensor_tensor(
                            Uf, sf2, float(D), sqf, ALU.mult, ALU.subtract)
                        lnvf = work.tile([P, LT], f32, tag="col", bufs=4,
                                         name=f"lnvf{ub}")
                        nc.scalar.activation(lnvf, Uf, AF.Ln, bias=eps_c,
                                             scale=1.0 / (D * D))
                        a_f = work.tile([P, LT], f32r, tag="col", bufs=4,
                                        name=f"af{ub}")
                        nc.scalar.activation(a_f, lnvf, AF.Exp, scale=-0.5)
                        if SUB < 4:
                            nc.vector.memset(fT[:, 0:3, :], 0.01)
                            return x1n
                        # columns -> row -> broadcast -> fT = x1T * a
                        ps_t = psum.tile([1, L], f32, tag="stat", bufs=1,
                                         name=f"pst{ub}")
                        for lt in range(LT):
                            nc.tensor.matmul(ps_t[0:1, lt * P:(lt + 1) * P],
                                             a_f[:, lt:lt + 1], ident,
                                             start=True, stop=True,
                                             skip_group_check=True)
                        a_row = work.tile([1, L], f32r, tag="row", bufs=2,
                                          name=f"arow{ub}")
                        nc.vector.tensor_copy(a_row, ps_t)
                        ps_ab = psum.tile([P, L], f32, tag="work", bufs=3,
                                          name=f"psab{ub}")
                        nc.tensor.matmul(ps_ab, ones_row, a_row,
                                         start=True, stop=True)
                        for kt in range(DT):
                            nc.vector.tensor_tensor(fT[:, kt, :],
                                                    x1T[:, kt, :], ps_ab,
                                                    ALU.mult)
                        return x1n

                    def epi3(x1n, b=b):
                        ub = f"b{b}"
                        if STOP < 5:
                            nc.sync.dma_start(
                                y[b].rearrange("(t p) d -> p t d", p=P), x1n)
                            return
                        f2 = work.tile([P, FFT, L], fp8, tag="f2", bufs=1,
                                       name=f"f2{ub}")
                        for mt in range(FFT):
                            ps_a = psum.tile([P, L], f32, tag="work", bufs=3,
                                             name=f"pfa{mt}{ub}")
                            ps_g = psum.tile([P, L], f32, tag="work", bufs=3,
                                             name=f"pfg{mt}{ub}")
                            for ks in PAIRS:
                                nc.tensor.matmul(
                                    ps_a,
                                    fw1_sb[:, ks[0]:ks[1],
                                           mt * P:(mt + 1) * P],
                                    fT[:, ks[0]:ks[1], :],
                                    start=(ks[0] == 0), stop=(ks[0] == 2),
                                    perf_mode=DR)
                            for ks in PAIRS:
                                nc.tensor.matmul(
                                    ps_g,
                                    fw1_sb[:, ks[0]:ks[1],
                                           (FFT + mt) * P:(FFT + mt + 1) * P],
                                    fT[:, ks[0]:ks[1], :],
                                    start=(ks[0] == 0), stop=(ks[0] == 2),
                                    perf_mode=DR)
                            g_sb = work.tile([P, L], bf16, tag="g", bufs=2,
                                             name=f"g{mt}{ub}")
                            nc.scalar.activation(g_sb, ps_g, gelu_af,
                                                 bias=bug_sb[:, mt:mt + 1],
                                                 scale=colc["ccgs"])
                            nc.vector.scalar_tensor_tensor(
                                f2[:, mt, :], ps_a, bua_sb[:, mt:mt + 1],
                                g_sb, ALU.add, ALU.mult)
                        y_sb = work.tile([P, LT, D], f32, tag="y", bufs=1,
                                         name=f"y{ub}")
                        for lt in range(LT):
                            ps = psum.tile([P, D], f32, tag="work", bufs=3,
                                           name=f"py{lt}{ub}")
                            for kk in range(0, FFT, 2):
                                nc.tensor.matmul(
                                    ps,
                                    f2[:, kk:kk + 2, lt * P:(lt + 1) * P],
                                    fw2_sb[:, kk:kk + 2, :],
                                    start=(kk == 0), stop=(kk == FFT - 2),
                                    perf_mode=DR)
                            nc.vector.scalar_tensor_tensor(
                                y_sb[:, lt, :], ps, colc["ccy"],
                                x1n[:, lt, :], ALU.mult, ALU.add)
                        nc.sync.dma_start(
                            y[b].rearrange("(t p) d -> p t d", p=P), y_sb)

                    pend = None
                    for c in range(nchunks):
                        sa = stage_a(c)
                        if pending is not None:
                            if c == 1 and pending_res is None:
                                pending_res = pending[0]()
                            elif c == 3:
                                flush_pending()
                        if pend is not None:
                            stage_b(pend[0], *pend[1])
                        pend = (c, sa)
                    stage_b(pend[0], *pend[1])
                    outn_b = epi1()
                    pending = (lambda o=outn_b, e2=epi2: e2(o), epi3)
                    pending_res = None
                flush_pending()

            _run()

    nc.compile()
    return nc


def _pow2(x):
    return float(2.0 ** np.round(np.log2(max(float(x), 1e-30))))


def host_prep(inputs, n_points=N_FULL):
    """Fold LN/means/biases into weights, pick fp8 scales, build in_maps."""
    f = lambda a: np.ascontiguousarray(np.asarray(a), dtype=np.float32)
    e4 = ml_dtypes.float8_e4m3fn
    x = f(inputs["x"])[:, :n_points, :]
    query = f(inputs["query"])[0]  # [L, D]

    W0, b0 = f(inputs["mlp_w0"]), f(inputs["mlp_b0"])
    W1, b1 = f(inputs["mlp_w1"]), f(inputs["mlp_b1"])
    W2, b2 = f(inputs["mlp_w2"]), f(inputs["mlp_b2"])
    W3, b3 = f(inputs["mlp_w3"]), f(inputs["mlp_b3"])

    # query path (batch independent, exact)
    gq, bq = f(inputs["ln_q_g"]), f(inputs["ln_q_b"])
    m = query.mean(-1, keepdims=True)
    v = query.var(-1, keepdims=True)
    qn = (query - m) / np.sqrt(v + EPS) * gq + bq
    q = qn @ f(inputs["wq"])  # [L, D]

    gctx, bctx = f(inputs["ln_ctx_g"]), f(inputs["ln_ctx_b"])
    wkv = f(inputs["wkv"])
    gwk = wkv[:, :D] * gctx[:, None]
    gwv = wkv[:, D:] * gctx[:, None]
    bv_const = bctx @ wkv[:, D:]           # beta @ wv

    wq2 = gwk @ q.T                        # [D, L]
    wq2c = wq2 - wq2.mean(0, keepdims=True)
    gwvc = gwv - gwv.mean(0, keepdims=True)
    W3q = W3 @ wq2c                        # [D, L]
    W3v = W3 @ gwvc                        # [D, D]
    bq_row = b3 @ wq2c                     # [L]
    bvv_row = b3 @ gwvc                    # [D]
    W3rs = W3.sum(1)                       # [D]

    # ---- sampled forward for activation-scale selection ----
    xs = x[0, :: max(1, n_points // 2048), :]
    relu = lambda a: np.maximum(a, 0.0)
    h0s = relu(xs @ W0 + b0)
    h1s = relu(h0s @ W1 + b1)
    h2s = relu(h1s @ W2 + b2)
    ctxs = h2s @ W3 + b3
    ms = ctxs.mean(-1, keepdims=True)
    vs = ctxs.var(-1, keepdims=True)
    a_s = 1.0 / np.sqrt(vs + EPS)
    v_s = (ctxs @ gwvc) * a_s              # sampled v minus bias
    logit = (ctxs @ wq2c) * a_s * SCALE    # [ns, L]
    attn = np.exp(logit - logit.max(0, keepdims=True))
    attn = attn / attn.sum(0, keepdims=True)
    out_s = attn.T @ (v_s + bv_const)      # [L, D]
    x1s = out_s @ f(inputs["wo"]) + f(inputs["bo"]) + query
    gff, bff = f(inputs["ln_ff_g"]), f(inputs["ln_ff_b"])
    mf = x1s.mean(-1, keepdims=True)
    vf = x1s.var(-1, keepdims=True)
    fns = (x1s - mf) / np.sqrt(vf + EPS)
    fw1_ = f(inputs["ff_w1"])
    gw1 = fw1_ * gff[:, None]
    bu_full = f(inputs["ff_b1"]) + bff @ fw1_
    gw1c = gw1 - gw1.mean(0, keepdims=True)
    aside_s = fns @ gw1c[:, :FF] + bu_full[:FF]
    gside_s = fns @ gw1c[:, FF:] + bu_full[FF:]
    from scipy.special import erf as _erf  # noqa
    gel_s = gside_s * 0.5 * (1.0 + _erf(gside_s / math.sqrt(2)))
    f2s = aside_s * gel_s

    sd = lambda a: max(float(np.std(a)), 1e-12)
    s0 = _pow2(1.0 / sd(h0s))
    s1 = _pow2(1.0 / sd(h1s))
    s2 = _pow2(1.0 / sd(h2s))
    ksq = _pow2(1.5 / sd(ctxs))
    k8v = _pow2(1.0 / sd(v_s))

    wsig = 0.25
    kw = lambda Wm: _pow2(wsig / sd(Wm))

    def aug(Wm, brow):
        M = Wm.shape[1] if Wm.ndim == 2 else 1
        out = np.zeros((P * KT, M), np.float32)
        out[:D] = Wm.reshape(D, M)
        out[D] = brow
        return out

    c1b = kw(W1 * (s1 / s0))
    W1p = aug(W1 * (s1 / s0) * c1b, b1 * s1 * c1b)
    c2b = kw(W2 * (s2 / s1))
    W2p = aug(W2 * (s2 / s1) * c2b, b2 * s2 * c2b)
    c3b = kw(W3 * (ksq / s2))
    W3p = aug(W3 * (ksq / s2) * c3b, b3 * ksq * c3b)
    kq = kw(W3q / s2)
    W3qp = aug(W3q * (kq / s2), bq_row * kq)
    kv = kw(W3v / s2)
    W3vp = aug(W3v * (kv / s2), bvv_row * kv)
    km = kw(W3rs)
    w3sp = aug((W3rs * km)[:, None], 0.0)
    ccb3_v = km * s2 * float(b3.sum())  # S1 bias, added in column math

    # stats: S1 = km*s2*D*m; sq = (ksq*ctx)^2 (csq = 1/c3b inside Square)
    kvar = (km * s2 * D) ** 2
    cc1_v = kvar / (D * ksq * ksq)
    clnv_v = 1.0 / kvar
    cbe_v = math.log(SCALE / (kq * s2)) + math.log(s2)  # = log(SCALE/kq)
    cbe_v = math.log(SCALE / kq)
    ccvr_v = (kq * k8v) / (SCALE * kv)

    # FF: fw1a scaled by kfa (weight range, f2 range); gelu side by kg
    kfa = kw(gw1c[:, :FF])
    while kfa * sd(f2s) > 8.0:
        kfa *= 0.5
    while kfa * sd(f2s) < 0.25:
        kfa *= 2.0
    kg = kw(gw1c[:, FF:])
    fw1p = np.zeros((P * KT, FF2), np.float32)
    fw1p[:D, :FF] = gw1c[:, :FF] * kfa
    fw1p[:D, FF:] = gw1c[:, FF:] * kg
    fw2_ = f(inputs["ff_w2"])
    kf2 = kw(fw2_ / kfa)
    fw2p = fw2_ * (kf2 / kfa)

    ccgs_v = 1.0 / kg
    ccy_v = 1.0 / kf2
    bua_v = np.ascontiguousarray((bu_full[:FF] * kfa).reshape(FFT, P).T)
    bug_v = np.ascontiguousarray(bu_full[FF:].reshape(FFT, P).T)

    bo_p = f(inputs["bo"]) + bv_const @ f(inputs["wo"])
    lqbT_v = np.ascontiguousarray(query.T + bo_p[:, None])
    lqbn_v = query + bo_p[None, :] + f(inputs["ff_b2"])[None, :]

    col = lambda val: np.full((P, 1), val, np.float32)
    common = {
        "w0": np.ascontiguousarray(
            np.concatenate([W0 * s0, (b0 * s0)[None, :]], 0)),
        "w1": W1p.astype(e4), "w2": W2p.astype(e4), "w3": W3p.astype(e4),
        "w3q": W3qp.astype(e4), "w3v": W3vp.astype(e4),
        "w3s": w3sp.astype(e4),
        "wo": f(inputs["wo"]) * (1.0 / k8v),
        "fw1": fw1p.astype(e4), "fw2": fw2p.astype(e4),
        "lqbT": lqbT_v, "lqbn": lqbn_v,
        "ident": np.eye(P, dtype=np.float32),
        "onesr": np.ones((1, P), np.float32),
        "ones16": np.concatenate(
            [np.ones((P * KT, 1), np.float32),
             np.zeros((P * KT, 15), np.float32)], 1).astype(e4),
        "padh": np.concatenate(
            [np.ones((1, CHUNK), np.float32),
             np.zeros((P - 1, CHUNK), np.float32)], 0).astype(e4),
        "padz": np.zeros((P, CHUNK), e4),
        "ch1": col(1.0 / c1b), "ch2": col(1.0 / c2b),
        "csq": col(1.0 / c3b),
        "cc1": col(cc1_v), "clnv": col(clnv_v), "cbe": col(cbe_v),
        "ccvr": col(ccvr_v), "ccgs": col(ccgs_v), "ccy": col(ccy_v),
        "ccb3": col(ccb3_v),
        "bua": bua_v, "bug": bug_v,
    }
    in_maps = []
    for c in range(NCORES):
        xs_ = x[c * BPC:(c + 1) * BPC]  # [BPC, n, 3]
        xTs = np.concatenate(
            [xs_.transpose(0, 2, 1),
             np.ones((BPC, 1, n_points), np.float32)], axis=1)
        in_maps.append({"xT": np.ascontiguousarray(xTs), **common})
    return in_maps


_NC_CACHE = {}


def get_nc(n_points=N_FULL):
    if n_points not in _NC_CACHE:
        _NC_CACHE[n_points] = build_nc(n_points)
    return _NC_CACHE[n_points]


def kernel(**inputs):
    from concourse.bass_utils import run_bass_kernel_spmd
    nc = get_nc(N_FULL)
    in_maps = host_prep(inputs, N_FULL)
    res = run_bass_kernel_spmd(nc, in_maps, core_ids=list(range(NCORES)))
    y = np.concatenate([r["y"] for r in res.results], axis=0)
    return y.astype(np.float32)


# revision 28
# speedup vs baseline: 3.9691x; 3.9691x over previous
"""Trainium2 Bass kernel for nn_PointEncoder (B=16, N=8192, L=512, D=384).

Sharding: data-parallel over batch, 2 batches per NeuronCore x 8 cores,
no collectives; full inputs sharded / outputs gathered on host.

v2 design (fp8e4 DoubleRow matmuls at 2x PE throughput):
  * MLP layers 1-3, ctx stats, scores, V, attn@V, the softmax
    denominator and the whole GEGLU FF all run as fp8 DoubleRow
    matmuls (0.5 cyc/row).  K dims are padded to 4x128; the pad
    subtile of each h tile carries a ones-row so every layer bias
    rides its matmul for free (works for arbitrary biases).
  * LayerNorm mean subtraction is folded into column-centered weights
    on the host (exact identity).  Only sum / sum-of-squares stats are
    computed on chip, directly in per-point column form via tiny
    DoubleRow matmuls; 1/sigma is applied through the ACT scale
    operand of the softmax exp and an ACT Copy-scale on V.
  * W3 (the linear last MLP layer) is folded into the score/V weights;
    ctx itself is only materialized as its square (for the variance)
    straight out of PSUM.
  * exp uses a constant -1 logit shift (cancels in softmax exactly) to
    bound fp8 magnitudes; logits are provably small, no max needed.
  * All data-dependent scale-compensation constants stream in as tiny
    column tensors, so the program is input-independent and compiles
    exactly once.

Engine split per 512-point chunk: PE ~20 DoubleRow + 5 f32r matmuls;
DVE: relu(h0,h1,h2) out of PSUM; ACT: square(ctx), exp x4, V-scale x4;
Pool (GpSimd, SBUF-only): tiny column math.  The per-batch epilogue
(attn normalize, output projections, FF stats from x1n rows, fp8
GEGLU) is software-pipelined across the next batch's chunk stream.
"""

import math
import numpy as np
import ml_dtypes

import concourse.bass as bass
import concourse.tile as tile
import concourse.mybir as mybir
from concourse import bacc

P = 128
B, N_FULL, L, D = 16, 8192, 512, 384
FF = 4 * D  # 1536
FF2 = 2 * FF  # 3072
DT = D // P  # 3
KT = 4      # padded K subtiles for D-contractions
LT = L // P  # 4
FFT = FF // P  # 12
CHUNK = 512
CT = CHUNK // P  # 4
NCORES = 8
BPC = B // NCORES  # 2

f32 = mybir.dt.float32
f32r = mybir.dt.float32r
bf16 = mybir.dt.bfloat16
fp8 = mybir.dt.float8e4
AF = mybir.ActivationFunctionType
ALU = mybir.AluOpType
DR = mybir.MatmulPerfMode.DoubleRow

EPS = 1e-5
SCALE = 1.0 / math.sqrt(D)
ESHIFT = -1.0  # constant logit shift inside exp; cancels in softmax

_tables_patched = False


def _patch_act_tables():
    """Steer the table chooser to 'natural_log_exp_and_others' (ln, exp,
    relu, square, copy) so the chunk stream needs no table swaps; only the
    epilogue Gelu block loads its own set."""
    global _tables_patched
    if _tables_patched:
        return
    from concourse import hw_specs, bacc as _bacc
    orig = hw_specs.get_activation_tables

    def patched(arch):
        t = dict(orig(arch))
        if "natural_log_exp_and_others" in t:
            if "exp_and_others" in t:
                t["exp_and_others"] = t["exp_and_others"] - {AF.Exp}
            if "natural_log" in t:
                t["natural_log"] = t["natural_log"] - {AF.Ln}
        return t

    _bacc.get_activation_tables = patched
    _tables_patched = True


def build_nc(n_points=N_FULL, bpc=BPC, gelu_af=None, repeat=None):
    import os
    STOP = int(os.environ.get("KSTOP", "5"))
    SUB = int(os.environ.get("KSUB", "9"))
    if gelu_af is None:
        gelu_af = AF.Gelu
    nchunks = n_points // CHUNK
    _patch_act_tables()
    nc = bacc.Bacc("TRN2", target_bir_lowering=False, debug=False,
                   enable_asserts=False)

    def di(name, shape, dtype=f32):
        return nc.dram_tensor(name, list(shape), dtype,
                              kind="ExternalInput").ap()

    xT = di("xT", [bpc, 4, n_points], f32r)     # row 3 = ones
    w0 = di("w0", [4, D], f32r)                 # [W0; b0] * s0
    w1 = di("w1", [P * KT, D], fp8)             # bias row at K=384, pad 0
    w2 = di("w2", [P * KT, D], fp8)
    w3 = di("w3", [P * KT, D], fp8)             # ctx (squared) path
    w3q = di("w3q", [P * KT, L], fp8)           # W3 @ centered wq2
    w3v = di("w3v", [P * KT, D], fp8)           # W3 @ centered gwv
    w3s = di("w3s", [P * KT, 1], fp8)           # ctx row-sum weights
    wo = di("wo", [D, D], f32r)                 # wo / k8v
    fw1 = di("fw1", [P * KT, FF2], fp8)         # centered, gain-folded
    fw2 = di("fw2", [FF, D], fp8)
    lqbT = di("lqbT", [D, L])                   # lq.T + bo'
    lqbn = di("lqbn", [L, D])                   # lq + bo' + fb2
    ident_d = di("ident", [P, P])
    onesr_d = di("onesr", [1, P])
    ones16_d = di("ones16", [P * KT, 16], fp8)  # col 0 = 1 (16-wide for DR)
    padh_d = di("padh", [P, CHUNK], fp8)        # row 0 = 1, rest 0
    padz_d = di("padz", [P, CHUNK], fp8)        # zeros
    # data-dependent per-partition constants (columns)
    cols_d = {n: di(n, [P, 1]) for n in
              ["ch1", "ch2", "csq", "cc1", "clnv", "cbe", "ccvr",
               "ccgs", "ccy", "ccb3"]}
    bua = di("bua", [P, FFT])    # kfa * bu_a, [p, mt]
    bug = di("bug", [P, FFT])    # bu_g, [p, mt]
    y = nc.dram_tensor("y", [bpc, L, D], f32, kind="ExternalOutput").ap()

    with tile.TileContext(nc) as tc:
        with tc.tile_pool(name="singles", bufs=1) as singles, \
             tc.tile_pool(name="work", bufs=1) as work, \
             tc.tile_pool(name="psum", bufs=1, space="PSUM") as psum:

            # ---------------- load params ----------------
            def ld(name, shape, dtype, src, eng=None):
                t = singles.tile(shape, dtype, name=name)
                (eng or nc.sync).dma_start(t, src)
                return t

            r4 = lambda a: a.rearrange("(t p) m -> p t m", p=P)

            xT_pre = work.tile([4, CHUNK], f32r, tag="xT", bufs=2,
                               name="xT_pre")
            nc.sync.dma_start(xT_pre, xT[0, :, 0:CHUNK])
            w0_sb = ld("w0_sb", [4, D], f32r, w0)
            w1_sb = ld("w1_sb", [P, KT, D], fp8, r4(w1))
            w2_sb = ld("w2_sb", [P, KT, D], fp8, r4(w2))
            w3_sb = ld("w3_sb", [P, KT, D], fp8, r4(w3))
            w3q_sb = ld("w3q_sb", [P, KT, L], fp8, r4(w3q))
            w3v_sb = ld("w3v_sb", [P, KT, D], fp8, r4(w3v))
            w3s_sb = ld("w3s_sb", [P, KT, 1], fp8, r4(w3s))
            colc = {n: ld(n + "_sb", [P, 1], f32, cols_d[n])
                    for n in cols_d}
            g = nc.gpsimd
            wo_sb = ld("wo_sb", [P, DT, D], f32r, r4(wo), eng=g)
            fw1_sb = ld("fw1_sb", [P, KT, FF2], fp8, r4(fw1), eng=g)
            fw2_sb = ld("fw2_sb", [P, FFT, D], fp8, r4(fw2), eng=g)
            lqbT_sb = ld("lqbT_sb", [P, DT, L], f32, r4(lqbT), eng=g)
            lqbn_sb = ld("lqbn_sb", [P, LT, D], f32,
                         lqbn.rearrange("(t p) d -> p t d", p=P), eng=g)
            bua_sb = ld("bua_sb", [P, FFT], f32, bua, eng=g)
            bug_sb = ld("bug_sb", [P, FFT], f32, bug, eng=g)
            ident = ld("ident_sb", [P, P], f32r, ident_d, eng=g)
            ones_row = ld("ones_row", [1, P], f32r, onesr_d, eng=g)
            ones16 = ld("ones16", [P, KT, 16], fp8, r4(ones16_d), eng=g)

            eps_c = singles.tile([P, 1], f32, name="eps_c")
            nc.vector.memset(eps_c, EPS)
            neg1_c = singles.tile([P, 1], f32, name="neg1_c")
            nc.vector.memset(neg1_c, ESHIFT)

            # persistent double-buffered h tiles; pad subtile 3 is zero with
            # a ones-row at partition 0 (K row 384) to carry biases.
            def padded_pair(name, pad_src):
                ts = []
                for i in range(3):
                    t = singles.tile([P, KT, CHUNK], fp8, name=f"{name}{i}")
                    nc.gpsimd.dma_start(t[:, 3, :], pad_src)
                    ts.append(t)
                return ts

            h0b = padded_pair("h0", padh_d)
            h1b = padded_pair("h1", padh_d)
            h2b = padded_pair("h2", padh_d)
            sqb = padded_pair("sq", padz_d)

            fT = singles.tile([P, KT, L], fp8, name="fT")
            nc.gpsimd.dma_start(fT[:, 3, :], padz_d)

            PAIRS = ((0, 2), (2, 4))

            def _run():
                pending = None       # (epi2_closure, epi3_closure)
                pending_res = None

                def flush_pending():
                    nonlocal pending, pending_res
                    if pending is not None:
                        if pending_res is None:
                            pending_res = pending[0]()
                        pending[1](pending_res)
                        pending = None
                        pending_res = None

                batches = []
                for b in range(bpc):
                    # 3 banks attn accumulators + bank 3 = den row
                    acc = psum.tile([P, KT, L], f32, tag="acc", name=f"acc{b}")

                    def stage_a(c, hi, b=b, acc=acc):
                        uid = f"{b}_{c}"
                        if b == 0 and c == 0:
                            xT_c = xT_pre
                        else:
                            xT_c = work.tile([4, CHUNK], f32r, tag="xT",
                                             bufs=2, name=f"xT{uid}")
                            nc.sync.dma_start(
                                xT_c, xT[b, :, c * CHUNK:(c + 1) * CHUNK])
                        h0, h1, h2, sq = h0b[hi], h1b[hi], h2b[hi], sqb[hi]
                        # L0 (f32r, K=4): relu on DVE
                        for mt in range(DT):
                            ps = psum.tile([P, CHUNK], f32, tag="work",
                                           bufs=4, name=f"ps0{mt}_{uid}")
                            nc.tensor.matmul(ps, w0_sb[:, mt * P:(mt + 1) * P],
                                             xT_c, start=True, stop=True)
                            nc.vector.tensor_scalar(
                                out=h0[:, mt, :], in0=ps, scalar1=0.0,
                                scalar2=None, op0=ALU.max)

                        def mm_dr(ps_out, w_sb, mt, rhs):
                            for ks in PAIRS:
                                nc.tensor.matmul(
                                    ps_out,
                                    w_sb[:, ks[0]:ks[1], mt * P:(mt + 1) * P],
                                    rhs[:, ks[0]:ks[1], :],
                                    start=(ks[0] == 0), stop=(ks[0] == 2),
                                    perf_mode=DR)

                        # L1, L2: relu via DVE TSP (mult comp-scale, max 0)
                        for li, (w_sb, hin, hout, cname) in enumerate(
                                ((w1_sb, h0, h1, "ch1"),
                                 (w2_sb, h1, h2, "ch2"))):
                            for mt in range(DT):
                                ps = psum.tile([P, CHUNK], f32, tag="pa",
                                               bufs=2,
                                               name=f"ps{li + 1}{mt}_{uid}")
                                mm_dr(ps, w_sb, mt, hin)
                                nc.vector.tensor_scalar(
                                    out=hout[:, mt, :], in0=ps,
                                    scalar1=colc[cname], scalar2=0.0,
                                    op0=ALU.mult, op1=ALU.max)
                        # L3 -> ctx materialized only as its square (ACT)
                        for mt in range(DT):
                            ps = psum.tile([P, CHUNK], f32, tag="work",
                                           bufs=4, name=f"ps3{mt}_{uid}")
                            mm_dr(ps, w3_sb, mt, h2)
                            nc.scalar.activation(sq[:, mt, :], ps, AF.Square,
                                                 scale=colc["csq"])

                        # per-point stats (S1 ~ sum ctx, S2 ~ sum ctx^2)
                        st_ps = psum.tile([P, 2, CT], f32, tag="work",
                                          bufs=4, name=f"st{uid}")
                        for jt in range(CT):
                            for kt in range(DT):
                                nc.tensor.matmul(
                                    st_ps[:, 0, jt:jt + 1],
                                    h2[:, kt, jt * P:(jt + 1) * P],
                                    w3s_sb[:, kt, :],
                                    start=(kt == 0), stop=(kt == DT - 1),
                                    skip_group_check=True)
                                nc.tensor.matmul(
                                    st_ps[:, 1, jt:jt + 1],
                                    sq[:, kt, jt * P:(jt + 1) * P],
                                    ones16[:, kt, 0:1],
                                    start=(kt == 0), stop=(kt == DT - 1),
                                    skip_group_check=True)
                        s1c = work.tile([P, CT], f32, tag="col", bufs=4,
                                        name=f"s1c{uid}")
                        nc.vector.tensor_scalar(
                            out=s1c, in0=st_ps[:, 0, :],
                            scalar1=colc["ccb3"], scalar2=None, op0=ALU.add)
                        sq1 = work.tile([P, CT], f32, tag="col", bufs=4,
                                        name=f"sq1{uid}")
                        nc.vector.tensor_tensor(sq1, s1c, s1c, ALU.mult)
                        U = work.tile([P, CT], f32, tag="col", bufs=4,
                                      name=f"U{uid}")
                        nc.vector.scalar_tensor_tensor(
                            U, st_ps[:, 1, :], colc["cc1"], sq1,
                            ALU.mult, ALU.subtract)
                        return hi, U

                    def stage_a2(c, U, b=b):
                        uid = f"{b}_{c}"
                        lnv = work.tile([P, CT], f32, tag="col", bufs=4,
                                        name=f"lnv{uid}")
                        nc.scalar.activation(lnv, U, AF.Ln, bias=eps_c,
                                             scale=colc["clnv"])
                        a_e = work.tile([P, CT], f32, tag="acol", bufs=3,
                                        name=f"ae{uid}")
                        nc.scalar.activation(a_e, lnv, AF.Exp,
                                             bias=colc["cbe"], scale=-0.5)
                        a_v = work.tile([P, CT], f32, tag="acol", bufs=3,
                                        name=f"av{uid}")
                        nc.vector.tensor_scalar(
                            out=a_v, in0=a_e, scalar1=colc["ccvr"],
                            scalar2=None, op0=ALU.mult)
                        return a_e, a_v

                    def stage_b1(c, hi, a_e, a_v, b=b):
                        if STOP < 2:
                            return c, None, None
                        uid = f"{b}_{c}"
                        h2 = h2b[hi]
                        v_t = work.tile([P, CT, D], fp8, tag="v", bufs=3,
                                        name=f"v{uid}")
                        e_t = work.tile([P, CT, L], fp8, tag="e", bufs=3,
                                        name=f"e{uid}")
                        for jt in range(CT):
                            psv = psum.tile([P, D], f32, tag="work", bufs=4,
                                            name=f"psv{jt}_{uid}")
                            for ks in PAIRS:
                                nc.tensor.matmul(
                                    psv,
                                    h2[:, ks[0]:ks[1], jt * P:(jt + 1) * P],
                                    w3v_sb[:, ks[0]:ks[1], :],
                                    start=(ks[0] == 0), stop=(ks[0] == 2),
                                    perf_mode=DR)
                            nc.scalar.activation(v_t[:, jt, :], psv, AF.Copy,
                                                 scale=a_v[:, jt:jt + 1])
                            pss = psum.tile([P, L], f32, tag="work", bufs=4,
                                            name=f"pss{jt}_{uid}")
                            for ks in PAIRS:
                                nc.tensor.matmul(
                                    pss,
                                    h2[:, ks[0]:ks[1], jt * P:(jt + 1) * P],
                                    w3q_sb[:, ks[0]:ks[1], :],
                                    start=(ks[0] == 0), stop=(ks[0] == 2),
                                    perf_mode=DR)
                            nc.scalar.activation(e_t[:, jt, :], pss, AF.Exp,
                                                 bias=neg1_c,
                                                 scale=a_e[:, jt:jt + 1])
                        return c, v_t, e_t

                    def stage_b2(c, v_t, e_t, b=b, acc=acc):
                        if STOP < 2 or v_t is None:
                            return
                        first, last = (c == 0), (c == nchunks - 1)
                        for pi, ks in enumerate(PAIRS):
                            for mt in range(DT):
                                nc.tensor.matmul(
                                    acc[:, mt, :],
                                    v_t[:, ks[0]:ks[1], mt * P:(mt + 1) * P],
                                    e_t[:, ks[0]:ks[1], :],
                                    start=(first and pi == 0),
                                    stop=(last and pi == 1),
                                    perf_mode=DR, skip_group_check=True)
                            nc.tensor.matmul(
                                acc[0:1, 3, :],
                                ones16[:, ks[0]:ks[1], 0:1],
                                e_t[:, ks[0]:ks[1], :],
                                start=(first and pi == 0),
                                stop=(last and pi == 1),
                                perf_mode=DR, skip_group_check=True)

                    def epi1(b=b, acc=acc):
                        ub = f"b{b}"
                        if STOP < 3:
                            outn = work.tile([P, DT, L], f32r, tag="outn",
                                             bufs=1, name=f"outn{ub}")
                            nc.vector.memset(outn, 0.01)
                            return outn
                        rec = work.tile([1, L], f32r, tag="row", bufs=2,
                                        name=f"rec{ub}")
                        with nc.allow_low_precision("f32r is full fp32"):
                            nc.vector.reciprocal(rec, acc[0:1, 3, :])
                        ps_rb = psum.tile([P, L], f32, tag="work", bufs=4,
                                          name=f"psrb{ub}")
                        nc.tensor.matmul(ps_rb, ones_row, rec,
                                         start=True, stop=True)
                        rb = work.tile([P, L], f32, tag="rb", bufs=1,
                                       name=f"rb{ub}")
                        nc.vector.tensor_copy(rb, ps_rb)
                        outn = work.tile([P, DT, L], f32r, tag="outn", bufs=1,
                                         name=f"outn{ub}")
                        for mt in range(DT):
                            nc.vector.tensor_tensor(outn[:, mt, :],
                                                    acc[:, mt, :], rb,
                                                    ALU.mult)
                        return outn

                    def epi2(outn, b=b):
                        ub = f"b{b}"
                        if STOP < 4:
                            x1n = work.tile([P, LT, D], f32, tag="x1n",
                                            bufs=1, name=f"x1n{ub}")
                            nc.vector.memset(x1n, 0.01)
                            return x1n
                        x1T = work.tile([P, DT, L], f32, tag="x1T", bufs=1,
                                        name=f"x1T{ub}")
                        for mt in range(DT):
                            ps = psum.tile([P, L], f32, tag="work", bufs=4,
                                           name=f"px1T{mt}{ub}")
                            for kt in range(DT):
                                nc.tensor.matmul(
                                    ps, wo_sb[:, kt, mt * P:(mt + 1) * P],
                                    outn[:, kt, :],
                                    start=(kt == 0), stop=(kt == DT - 1))
                            nc.vector.tensor_tensor(x1T[:, mt, :], ps,
                                                    lqbT_sb[:, mt, :], ALU.add)
                        x1n = work.tile([P, LT, D], f32, tag="x1n", bufs=1,
                                        name=f"x1n{ub}")
                        if SUB < 2:
                            nc.vector.memset(x1n, 0.01)
                            nc.vector.memset(fT[:, 0:3, :], 0.01)
                            return x1n
                        for lt in range(LT):
                            ps = psum.tile([P, D], f32, tag="work", bufs=4,
                                           name=f"px1n{lt}{ub}")
                            for kt in range(DT):
                                nc.tensor.matmul(
                                    ps, outn[:, kt, lt * P:(lt + 1) * P],
                                    wo_sb[:, kt, :],
                                    start=(kt == 0), stop=(kt == DT - 1))
                            nc.vector.tensor_tensor(x1n[:, lt, :], ps,
                                                    lqbn_sb[:, lt, :], ALU.add)
                        if SUB < 3:
                            nc.vector.memset(fT[:, 0:3, :], 0.01)
                            return x1n
                        # FF LN stats, per-latent column form from x1n
                        sf1 = work.tile([P, LT], f32, tag="col", bufs=4,
                                        name=f"sf1{ub}")
                        sf2 = work.tile([P, LT], f32, tag="col", bufs=4,
                                        name=f"sf2{ub}")
                        x1sq = work.tile([P, D], f32, tag="x1sq", bufs=2,
                                         name=f"x1sq{ub}")
                        for lt in range(LT):
                            nc.vector.tensor_reduce(
                                sf1[:, lt:lt + 1], x1n[:, lt, :],
                                mybir.AxisListType.X, ALU.add)
                            nc.gpsimd.tensor_tensor(x1sq, x1n[:, lt, :],
                                                    x1n[:, lt, :], ALU.mult)
                            nc.vector.tensor_reduce(
                                sf2[:, lt:lt + 1], x1sq,
                                mybir.AxisListType.X, ALU.add)
                        sqf = work.tile([P, LT], f32, tag="col", bufs=4,
                                        name=f"sqf{ub}")
                        nc.vector.tensor_tensor(sqf, sf1, sf1, ALU.mult)
                        Uf = work.tile([P, LT], f32, tag="col", bufs=4,
                                       name=f"Uf{ub}")
                        nc.vector.scalar_tensor_tensor(
                            Uf, sf2, float(D), sqf, ALU.mult, ALU.subtract)
                        lnvf = work.tile([P, LT], f32, tag="col", bufs=4,
                                         name=f"lnvf{ub}")
                        nc.scalar.activation(lnvf, Uf, AF.Ln, bias=eps_c,
                                             scale=1.0 / (D * D))
                        a_f = work.tile([P, LT], f32r, tag="col", bufs=4,
                                        name=f"af{ub}")
                        nc.scalar.activation(a_f, lnvf, AF.Exp, scale=-0.5)
                        if SUB < 4:
                            nc.vector.memset(fT[:, 0:3, :], 0.01)
                            return x1n
                        # columns -> row -> broadcast -> fT = x1T * a
                        ps_t = psum.tile([1, L], f32, tag="work", bufs=4,
                                         name=f"pst{ub}")
                        for lt in range(LT):
                            nc.tensor.matmul(ps_t[0:1, lt * P:(lt + 1) * P],
                                             a_f[:, lt:lt + 1], ident,
                                             start=True, stop=True,
                                             skip_group_check=True)
                        a_row = work.tile([1, L], f32r, tag="row", bufs=2,
                                          name=f"arow{ub}")
                        nc.vector.tensor_copy(a_row, ps_t)
                        ps_ab = psum.tile([P, L], f32, tag="work", bufs=4,
                                          name=f"psab{ub}")
                        nc.tensor.matmul(ps_ab, ones_row, a_row,
                                         start=True, stop=True)
                        for kt in range(DT):
                            nc.vector.tensor_tensor(fT[:, kt, :],
                                                    x1T[:, kt, :], ps_ab,
                                                    ALU.mult)
                        return x1n

                    def epi3(x1n, b=b):
                        ub = f"b{b}"
                        if STOP < 5:
                            nc.sync.dma_start(
                                y[b].rearrange("(t p) d -> p t d", p=P), x1n)
                            return
                        f2 = work.tile([P, FFT, L], fp8, tag="f2", bufs=1,
                                       name=f"f2{ub}")
                        for mt in range(FFT):
                            ps_a = psum.tile([P, L], f32, tag="work", bufs=4,
                                             name=f"pfa{mt}{ub}")
                            ps_g = psum.tile([P, L], f32, tag="work", bufs=4,
                                             name=f"pfg{mt}{ub}")
                            for ks in PAIRS:
                                nc.tensor.matmul(
                                    ps_a,
                                    fw1_sb[:, ks[0]:ks[1],
                                           mt * P:(mt + 1) * P],
                                    fT[:, ks[0]:ks[1], :],
                                    start=(ks[0] == 0), stop=(ks[0] == 2),
                                    perf_mode=DR)
                            for ks in PAIRS:
                                nc.tensor.matmul(
                                    ps_g,
                                    fw1_sb[:, ks[0]:ks[1],
                                           (FFT + mt) * P:(FFT + mt + 1) * P],
                                    fT[:, ks[0]:ks[1], :],
                                    start=(ks[0] == 0), stop=(ks[0] == 2),
                                    perf_mode=DR)
                            g_sb = work.tile([P, L], bf16, tag="g", bufs=2,
                                             name=f"g{mt}{ub}")
                            nc.scalar.activation(g_sb, ps_g, gelu_af,
                                                 bias=bug_sb[:, mt:mt + 1],
                                                 scale=colc["ccgs"])
                            nc.vector.scalar_tensor_tensor(
                                f2[:, mt, :], ps_a, bua_sb[:, mt:mt + 1],
                                g_sb, ALU.add, ALU.mult)
                        y_sb = work.tile([P, LT, D], f32, tag="y", bufs=1,
                                         name=f"y{ub}")
                        for lt in range(LT):
                            ps = psum.tile([P, D], f32, tag="work", bufs=4,
                                           name=f"py{lt}{ub}")
                            for kk in range(0, FFT, 2):
                                nc.tensor.matmul(
                                    ps,
                                    f2[:, kk:kk + 2, lt * P:(lt + 1) * P],
                                    fw2_sb[:, kk:kk + 2, :],
                                    start=(kk == 0), stop=(kk == FFT - 2),
                                    perf_mode=DR)
                            nc.vector.scalar_tensor_tensor(
                                y_sb[:, lt, :], ps, colc["ccy"],
                                x1n[:, lt, :], ALU.mult, ALU.add)
                        nc.sync.dma_start(
                            y[b].rearrange("(t p) d -> p t d", p=P), y_sb)

                    batches.append((stage_a, stage_a2, stage_b1,
                                    stage_b2, epi1, epi2, epi3))

                # ---- continuous cross-batch software pipeline ----
                b1q, b2q = [], []
                pend_age = 0

                def tick_pending():
                    nonlocal pending, pending_res, pend_age
                    if pending is not None:
                        pend_age += 1
                        if pend_age == 2 and pending_res is None:
                            pending_res = pending[0]()
                        elif pend_age >= 4:
                            flush_pending()

                def pump(drain=False):
                    nonlocal pending, pending_res, pend_age
                    r1 = None
                    if b1q and (drain or len(b1q) > 1):
                        SB1, SB2, E1, E2, E3, c, hi, a_e, a_v, last = \
                            b1q.pop(0)
                        v = SB1(c, hi, a_e, a_v)
                        r1 = (SB2, E1, E2, E3, v, last)
                    if b2q:
                        SB2, E1, E2, E3, v, last = b2q.pop(0)
                        SB2(*v)
                        if last:
                            outn_b = E1()
                            pending = (lambda o=outn_b, e2=E2: e2(o), E3)
                            pending_res = None
                            pend_age = 0
                    if r1 is not None:
                        b2q.append(r1)

                for gc in range(bpc * nchunks):
                    bb, c = divmod(gc, nchunks)
                    SA, SA2, SB1, SB2, E1, E2, E3 = batches[bb]
                    pump()
                    tick_pending()
                    hi, U = SA(c, gc % 3)
                    a_e, a_v = SA2(c, U)
                    b1q.append((SB1, SB2, E1, E2, E3, c, hi, a_e, a_v,
                                c == nchunks - 1))
                while b1q or b2q:
                    pump(drain=True)
                    tick_pending()
                flush_pending()

            if repeat is not None and repeat > 1:
                with tc.For_i(0, repeat, 1):
                    _run()
            else:
                _run()

    nc.compile()
    return nc


def _pow2(x):
    return float(2.0 ** np.round(np.log2(max(float(x), 1e-30))))


def host_prep(inputs, n_points=N_FULL):
    """Fold LN/means/biases into weights, pick fp8 scales, build in_maps."""
    f = lambda a: np.ascontiguousarray(np.asarray(a), dtype=np.float32)
    e4 = ml_dtypes.float8_e4m3fn
    x = f(inputs["x"])[:, :n_points, :]
    query = f(inputs["query"])[0]  # [L, D]

    W0, b0 = f(inputs["mlp_w0"]), f(inputs["mlp_b0"])
    W1, b1 = f(inputs["mlp_w1"]), f(inputs["mlp_b1"])
    W2, b2 = f(inputs["mlp_w2"]), f(inputs["mlp_b2"])
    W3, b3 = f(inputs["mlp_w3"]), f(inputs["mlp_b3"])

    # query path (batch independent, exact)
    gq, bq = f(inputs["ln_q_g"]), f(inputs["ln_q_b"])
    m = query.mean(-1, keepdims=True)
    v = query.var(-1, keepdims=True)
    qn = (query - m) / np.sqrt(v + EPS) * gq + bq
    q = qn @ f(inputs["wq"])  # [L, D]

    gctx, bctx = f(inputs["ln_ctx_g"]), f(inputs["ln_ctx_b"])
    wkv = f(inputs["wkv"])
    gwk = wkv[:, :D] * gctx[:, None]
    gwv = wkv[:, D:] * gctx[:, None]
    bv_const = bctx @ wkv[:, D:]           # beta @ wv

    wq2 = gwk @ q.T                        # [D, L]
    wq2c = wq2 - wq2.mean(0, keepdims=True)
    gwvc = gwv - gwv.mean(0, keepdims=True)
    W3q = W3 @ wq2c                        # [D, L]
    W3v = W3 @ gwvc                        # [D, D]
    bq_row = b3 @ wq2c                     # [L]
    bvv_row = b3 @ gwvc                    # [D]
    W3rs = W3.sum(1)                       # [D]

    # ---- sampled forward for activation-scale selection ----
    xs = x[0, :: max(1, n_points // 2048), :]
    relu = lambda a: np.maximum(a, 0.0)
    h0s = relu(xs @ W0 + b0)
    h1s = relu(h0s @ W1 + b1)
    h2s = relu(h1s @ W2 + b2)
    ctxs = h2s @ W3 + b3
    ms = ctxs.mean(-1, keepdims=True)
    vs = ctxs.var(-1, keepdims=True)
    a_s = 1.0 / np.sqrt(vs + EPS)
    v_s = (ctxs @ gwvc) * a_s              # sampled v minus bias
    logit = (ctxs @ wq2c) * a_s * SCALE    # [ns, L]
    attn = np.exp(logit - logit.max(0, keepdims=True))
    attn = attn / attn.sum(0, keepdims=True)
    out_s = attn.T @ (v_s + bv_const)      # [L, D]
    x1s = out_s @ f(inputs["wo"]) + f(inputs["bo"]) + query
    gff, bff = f(inputs["ln_ff_g"]), f(inputs["ln_ff_b"])
    mf = x1s.mean(-1, keepdims=True)
    vf = x1s.var(-1, keepdims=True)
    fns = (x1s - mf) / np.sqrt(vf + EPS)
    fw1_ = f(inputs["ff_w1"])
    gw1 = fw1_ * gff[:, None]
    bu_full = f(inputs["ff_b1"]) + bff @ fw1_
    gw1c = gw1 - gw1.mean(0, keepdims=True)
    aside_s = fns @ gw1c[:, :FF] + bu_full[:FF]
    gside_s = fns @ gw1c[:, FF:] + bu_full[FF:]
    _erf = np.vectorize(math.erf)
    gel_s = gside_s * 0.5 * (1.0 + _erf(gside_s / math.sqrt(2)))
    f2s = aside_s * gel_s

    sd = lambda a: max(float(np.std(a)), 1e-12)
    s0 = _pow2(1.0 / sd(h0s))
    s1 = _pow2(1.0 / sd(h1s))
    s2 = _pow2(1.0 / sd(h2s))
    ksq = _pow2(1.5 / sd(ctxs))
    k8v = _pow2(1.0 / sd(v_s))

    wsig = 0.25
    kw = lambda Wm: _pow2(wsig / sd(Wm))

    def aug(Wm, brow):
        M = Wm.shape[1] if Wm.ndim == 2 else 1
        out = np.zeros((P * KT, M), np.float32)
        out[:D] = Wm.reshape(D, M)
        out[D] = brow
        return out

    c1b = kw(W1 * (s1 / s0))
    W1p = aug(W1 * (s1 / s0) * c1b, b1 * s1 * c1b)
    c2b = kw(W2 * (s2 / s1))
    W2p = aug(W2 * (s2 / s1) * c2b, b2 * s2 * c2b)
    c3b = kw(W3 * (ksq / s2))
    W3p = aug(W3 * (ksq / s2) * c3b, b3 * ksq * c3b)
    kq = kw(W3q / s2)
    W3qp = aug(W3q * (kq / s2), bq_row * kq)
    kv = kw(W3v / s2)
    W3vp = aug(W3v * (kv / s2), bvv_row * kv)
    km = kw(W3rs)
    w3sp = aug((W3rs * km)[:, None], 0.0)
    ccb3_v = km * s2 * float(b3.sum())  # S1 bias, added in column math

    # stats: S1 = km*s2*D*m; sq = (ksq*ctx)^2 (csq = 1/c3b inside Square)
    kvar = (km * s2 * D) ** 2
    cc1_v = kvar / (D * ksq * ksq)
    clnv_v = 1.0 / kvar
    cbe_v = math.log(SCALE / (kq * s2)) + math.log(s2)  # = log(SCALE/kq)
    cbe_v = math.log(SCALE / kq)
    ccvr_v = (kq * k8v) / (SCALE * kv)

    # FF: fw1a scaled by kfa (weight range, f2 range); gelu side by kg
    kfa = kw(gw1c[:, :FF])
    while kfa * sd(f2s) > 8.0:
        kfa *= 0.5
    while kfa * sd(f2s) < 0.25:
        kfa *= 2.0
    kg = kw(gw1c[:, FF:])
    fw1p = np.zeros((P * KT, FF2), np.float32)
    fw1p[:D, :FF] = gw1c[:, :FF] * kfa
    fw1p[:D, FF:] = gw1c[:, FF:] * kg
    fw2_ = f(inputs["ff_w2"])
    kf2 = kw(fw2_ / kfa)
    fw2p = fw2_ * (kf2 / kfa)

    ccgs_v = 1.0 / kg
    ccy_v = 1.0 / kf2
    bua_v = np.ascontiguousarray((bu_full[:FF] * kfa).reshape(FFT, P).T)
    bug_v = np.ascontiguousarray(bu_full[FF:].reshape(FFT, P).T)

    bo_p = f(inputs["bo"]) + bv_const @ f(inputs["wo"])
    lqbT_v = np.ascontiguousarray(query.T + bo_p[:, None])
    lqbn_v = query + bo_p[None, :] + f(inputs["ff_b2"])[None, :]

    col = lambda val: np.full((P, 1), val, np.float32)
    common = {
        "w0": np.ascontiguousarray(
            np.concatenate([W0 * s0, (b0 * s0)[None, :]], 0)),
        "w1": W1p.astype(e4), "w2": W2p.astype(e4), "w3": W3p.astype(e4),
        "w3q": W3qp.astype(e4), "w3v": W3vp.astype(e4),
        "w3s": w3sp.astype(e4),
        "wo": f(inputs["wo"]) * (1.0 / k8v),
        "fw1": fw1p.astype(e4), "fw2": fw2p.astype(e4),
        "lqbT": lqbT_v, "lqbn": lqbn_v,
        "ident": np.eye(P, dtype=np.float32),
        "onesr": np.ones((1, P), np.float32),
        "ones16": np.concatenate(
            [np.ones((P * KT, 1), np.float32),
             np.zeros((P * KT, 15), np.float32)], 1).astype(e4),
        "padh": np.concatenate(
            [np.ones((1, CHUNK), np.float32),
             np.zeros((P - 1, CHUNK), np.float32)], 0).astype(e4),
        "padz": np.zeros((P, CHUNK), e4),
        "ch1": col(1.0 / c1b), "ch2": col(1.0 / c2b),
        "csq": col(1.0 / c3b),
        "cc1": col(cc1_v), "clnv": col(clnv_v), "cbe": col(cbe_v),
        "ccvr": col(ccvr_v), "ccgs": col(ccgs_v), "ccy": col(ccy_v),
        "ccb3": col(ccb3_v),
        "bua": bua_v, "bug": bug_v,
    }
    in_maps = []
    for c in range(NCORES):
        xs_ = x[c * BPC:(c + 1) * BPC]  # [BPC, n, 3]
        xTs = np.concatenate(
            [xs_.transpose(0, 2, 1),
             np.ones((BPC, 1, n_points), np.float32)], axis=1)
        in_maps.append({"xT": np.ascontiguousarray(xTs), **common})
    return in_maps


_NC_CACHE = {}


def get_nc(n_points=N_FULL):
    if n_points not in _NC_CACHE:
        _NC_CACHE[n_points] = build_nc(n_points)
    return _NC_CACHE[n_points]


def kernel(**inputs):
    from concourse.bass_utils import run_bass_kernel_spmd
    nc = get_nc(N_FULL)
    in_maps = host_prep(inputs, N_FULL)
    res = run_bass_kernel_spmd(nc, in_maps, core_ids=list(range(NCORES)))
    y = np.concatenate([r["y"] for r in res.results], axis=0)
    return y.astype(np.float32)


# revision 31
# speedup vs baseline: 4.3221x; 1.0889x over previous
"""Trainium2 Bass kernel for nn_PointEncoder (B=16, N=8192, L=512, D=384).

Sharding: data-parallel over batch, 2 batches per NeuronCore x 8 cores,
no collectives; full inputs sharded / outputs gathered on host.

v2 design (fp8e4 DoubleRow matmuls at 2x PE throughput):
  * MLP layers 1-3, ctx stats, scores, V, attn@V, the softmax
    denominator and the whole GEGLU FF all run as fp8 DoubleRow
    matmuls (0.5 cyc/row).  K dims are padded to 4x128; the pad
    subtile of each h tile carries a ones-row so every layer bias
    rides its matmul for free (works for arbitrary biases).
  * LayerNorm mean subtraction is folded into column-centered weights
    on the host (exact identity).  Only sum / sum-of-squares stats are
    computed on chip, directly in per-point column form via tiny
    DoubleRow matmuls; 1/sigma is applied through the ACT scale
    operand of the softmax exp and an ACT Copy-scale on V.
  * W3 (the linear last MLP layer) is folded into the score/V weights;
    ctx itself is only materialized as its square (for the variance)
    straight out of PSUM.
  * exp uses a constant -1 logit shift (cancels in softmax exactly) to
    bound fp8 magnitudes; logits are provably small, no max needed.
  * All data-dependent scale-compensation constants stream in as tiny
    column tensors, so the program is input-independent and compiles
    exactly once.

Engine split per 512-point chunk: PE ~20 DoubleRow + 5 f32r matmuls;
DVE: relu(h0,h1,h2) out of PSUM; ACT: square(ctx), exp x4, V-scale x4;
Pool (GpSimd, SBUF-only): tiny column math.  The per-batch epilogue
(attn normalize, output projections, FF stats from x1n rows, fp8
GEGLU) is software-pipelined across the next batch's chunk stream.
"""

import math
import numpy as np
import ml_dtypes

import concourse.bass as bass
import concourse.tile as tile
import concourse.mybir as mybir
from concourse import bacc

P = 128
B, N_FULL, L, D = 16, 8192, 512, 384
FF = 4 * D  # 1536
FF2 = 2 * FF  # 3072
DT = D // P  # 3
KT = 4      # padded K subtiles for D-contractions
LT = L // P  # 4
FFT = FF // P  # 12
CHUNK = 512
CT = CHUNK // P  # 4
NCORES = 8
BPC = B // NCORES  # 2

f32 = mybir.dt.float32
f32r = mybir.dt.float32r
bf16 = mybir.dt.bfloat16
fp8 = mybir.dt.float8e4
AF = mybir.ActivationFunctionType
ALU = mybir.AluOpType
DR = mybir.MatmulPerfMode.DoubleRow

EPS = 1e-5
SCALE = 1.0 / math.sqrt(D)
ESHIFT = -1.0  # constant logit shift inside exp; cancels in softmax

_tables_patched = False


def _patch_act_tables():
    """Steer the table chooser to 'natural_log_exp_and_others' (ln, exp,
    relu, square, copy) so the chunk stream needs no table swaps; only the
    epilogue Gelu block loads its own set."""
    global _tables_patched
    if _tables_patched:
        return
    from concourse import hw_specs, bacc as _bacc
    orig = hw_specs.get_activation_tables

    def patched(arch):
        t = dict(orig(arch))
        if "natural_log_exp_and_others" in t:
            if "exp_and_others" in t:
                t["exp_and_others"] = t["exp_and_others"] - {AF.Exp}
            if "natural_log" in t:
                t["natural_log"] = t["natural_log"] - {AF.Ln}
        return t

    _bacc.get_activation_tables = patched
    _tables_patched = True


def build_nc(n_points=N_FULL, bpc=BPC, gelu_af=None, repeat=None):
    import os
    STOP = int(os.environ.get("KSTOP", "5"))
    SUB = int(os.environ.get("KSUB", "9"))
    if gelu_af is None:
        gelu_af = AF.Gelu
    nchunks = n_points // CHUNK
    _patch_act_tables()
    nc = bacc.Bacc("TRN2", target_bir_lowering=False, debug=False,
                   enable_asserts=False)

    def di(name, shape, dtype=f32):
        return nc.dram_tensor(name, list(shape), dtype,
                              kind="ExternalInput").ap()

    xT = di("xT", [bpc, 4, n_points], f32r)     # row 3 = ones
    w0 = di("w0", [4, D], f32r)                 # [W0; b0] * s0
    w1 = di("w1", [P * KT, D], fp8)             # bias row at K=384, pad 0
    w2 = di("w2", [P * KT, D], fp8)
    w3 = di("w3", [P * KT, D], fp8)             # ctx (squared) path
    w3q = di("w3q", [P * KT, L], fp8)           # W3 @ centered wq2
    w3v = di("w3v", [P * KT, D], fp8)           # W3 @ centered gwv
    w3s = di("w3s", [P * KT, 1], fp8)           # ctx row-sum weights
    wo = di("wo", [D, D], f32r)                 # wo / k8v
    fw1 = di("fw1", [P * KT, FF2], fp8)         # centered, gain-folded
    fw2 = di("fw2", [FF, D], fp8)
    lqbT = di("lqbT", [D, L])                   # lq.T + bo'
    lqbn = di("lqbn", [L, D])                   # lq + bo' + fb2
    ident_d = di("ident", [P, P])
    onesr_d = di("onesr", [1, P])
    ones16_d = di("ones16", [P * KT, 16], fp8)  # col 0 = 1 (16-wide for DR)
    padh_d = di("padh", [P, CHUNK], fp8)        # row 0 = 1, rest 0
    padz_d = di("padz", [P, CHUNK], fp8)        # zeros
    # data-dependent per-partition constants (columns)
    cols_d = {n: di(n, [P, 1]) for n in
              ["ch1", "ch2", "csq", "cc1", "clnv", "cbe", "ccvr",
               "ccgs", "ccy", "ccb3"]}
    bua = di("bua", [P, FFT])    # kfa * bu_a, [p, mt]
    bug = di("bug", [P, FFT])    # bu_g, [p, mt]
    y = nc.dram_tensor("y", [bpc, L, D], f32, kind="ExternalOutput").ap()

    with tile.TileContext(nc) as tc:
        with tc.tile_pool(name="singles", bufs=1) as singles, \
             tc.tile_pool(name="work", bufs=1) as work, \
             tc.tile_pool(name="psum", bufs=1, space="PSUM") as psum:

            # ---------------- load params ----------------
            def ld(name, shape, dtype, src, eng=None):
                t = singles.tile(shape, dtype, name=name)
                (eng or nc.sync).dma_start(t, src)
                return t

            r4 = lambda a: a.rearrange("(t p) m -> p t m", p=P)

            xT_pre = work.tile([4, CHUNK], f32r, tag="xT", bufs=2,
                               name="xT_pre")
            nc.sync.dma_start(xT_pre, xT[0, :, 0:CHUNK])
            w0_sb = ld("w0_sb", [4, D], f32r, w0)
            w1_sb = ld("w1_sb", [P, KT, D], fp8, r4(w1))
            w2_sb = ld("w2_sb", [P, KT, D], fp8, r4(w2))
            w3_sb = ld("w3_sb", [P, KT, D], fp8, r4(w3))
            w3q_sb = ld("w3q_sb", [P, KT, L], fp8, r4(w3q))
            w3v_sb = ld("w3v_sb", [P, KT, D], fp8, r4(w3v))
            w3s_sb = ld("w3s_sb", [P, KT, 1], fp8, r4(w3s))
            colc = {n: ld(n + "_sb", [P, 1], f32, cols_d[n])
                    for n in cols_d}
            g = nc.gpsimd
            wo_sb = ld("wo_sb", [P, DT, D], f32r, r4(wo), eng=g)
            fw1_sb = ld("fw1_sb", [P, KT, FF2], fp8, r4(fw1), eng=g)
            fw2_sb = ld("fw2_sb", [P, FFT, D], fp8, r4(fw2), eng=g)
            lqbT_sb = ld("lqbT_sb", [P, DT, L], f32, r4(lqbT), eng=g)
            lqbn_sb = ld("lqbn_sb", [P, LT, D], f32,
                         lqbn.rearrange("(t p) d -> p t d", p=P), eng=g)
            bua_sb = ld("bua_sb", [P, FFT], f32, bua, eng=g)
            bug_sb = ld("bug_sb", [P, FFT], f32, bug, eng=g)
            ident = ld("ident_sb", [P, P], f32r, ident_d, eng=g)
            ones_row = ld("ones_row", [1, P], f32r, onesr_d, eng=g)
            ones16 = ld("ones16", [P, KT, 16], fp8, r4(ones16_d), eng=g)

            eps_c = singles.tile([P, 1], f32, name="eps_c")
            nc.vector.memset(eps_c, EPS)
            neg1_c = singles.tile([P, 1], f32, name="neg1_c")
            nc.vector.memset(neg1_c, ESHIFT)

            # persistent double-buffered h tiles; pad subtile 3 is zero with
            # a ones-row at partition 0 (K row 384) to carry biases.
            def padded_pair(name, pad_src):
                ts = []
                for i in range(3):
                    t = singles.tile([P, KT, CHUNK], fp8, name=f"{name}{i}")
                    nc.gpsimd.dma_start(t[:, 3, :], pad_src)
                    ts.append(t)
                return ts

            h0b = padded_pair("h0", padh_d)
            h1b = padded_pair("h1", padh_d)
            h2b = padded_pair("h2", padh_d)
            sqb = padded_pair("sq", padz_d)

            fT = singles.tile([P, KT, L], fp8, name="fT")
            nc.gpsimd.dma_start(fT[:, 3, :], padz_d)

            PAIRS = ((0, 2), (2, 4))

            def _run():
                pending = None       # (epi2_closure, epi3_closure)
                pending_res = None

                def flush_pending():
                    nonlocal pending, pending_res
                    if pending is not None:
                        if pending_res is None:
                            pending_res = pending[0]()
                        pending[1](pending_res)
                        pending = None
                        pending_res = None

                batches = []
                for b in range(bpc):
                    # 3 banks attn accumulators + bank 3 = den row
                    acc = psum.tile([P, KT, L], f32, tag="acc", name=f"acc{b}")

                    def stage_a(c, hi, b=b, acc=acc):
                        uid = f"{b}_{c}"
                        if b == 0 and c == 0:
                            xT_c = xT_pre
                        else:
                            xT_c = work.tile([4, CHUNK], f32r, tag="xT",
                                             bufs=2, name=f"xT{uid}")
                            nc.sync.dma_start(
                                xT_c, xT[b, :, c * CHUNK:(c + 1) * CHUNK])
                        h0, h1, h2, sq = h0b[hi], h1b[hi], h2b[hi], sqb[hi]
                        # L0 (f32r, K=4): relu on DVE
                        for mt in range(DT):
                            ps = psum.tile([P, CHUNK], f32, tag="work",
                                           bufs=4, name=f"ps0{mt}_{uid}")
                            nc.tensor.matmul(ps, w0_sb[:, mt * P:(mt + 1) * P],
                                             xT_c, start=True, stop=True)
                            nc.vector.tensor_scalar(
                                out=h0[:, mt, :], in0=ps, scalar1=0.0,
                                scalar2=None, op0=ALU.max)

                        def mm_dr(ps_out, w_sb, mt, rhs):
                            for ks in PAIRS:
                                nc.tensor.matmul(
                                    ps_out,
                                    w_sb[:, ks[0]:ks[1], mt * P:(mt + 1) * P],
                                    rhs[:, ks[0]:ks[1], :],
                                    start=(ks[0] == 0), stop=(ks[0] == 2),
                                    perf_mode=DR)

                        # L1, L2: relu via DVE TSP (mult comp-scale, max 0)
                        for li, (w_sb, hin, hout, cname) in enumerate(
                                ((w1_sb, h0, h1, "ch1"),
                                 (w2_sb, h1, h2, "ch2"))):
                            for mt in range(DT):
                                ps = psum.tile([P, CHUNK], f32, tag="pa",
                                               bufs=2,
                                               name=f"ps{li + 1}{mt}_{uid}")
                                mm_dr(ps, w_sb, mt, hin)
                                nc.vector.tensor_scalar(
                                    out=hout[:, mt, :], in0=ps,
                                    scalar1=colc[cname], scalar2=0.0,
                                    op0=ALU.mult, op1=ALU.max)
                        # L3 -> ctx materialized only as its square (ACT)
                        for mt in range(DT):
                            ps = psum.tile([P, CHUNK], f32, tag="work",
                                           bufs=4, name=f"ps3{mt}_{uid}")
                            mm_dr(ps, w3_sb, mt, h2)
                            nc.scalar.activation(sq[:, mt, :], ps, AF.Square,
                                                 scale=colc["csq"])

                        # per-point stats (S1 ~ sum ctx, S2 ~ sum ctx^2)
                        st_ps = psum.tile([P, 2, CT], f32, tag="work",
                                          bufs=4, name=f"st{uid}")
                        for jt in range(CT):
                            for kt in range(DT):
                                nc.tensor.matmul(
                                    st_ps[:, 0, jt:jt + 1],
                                    h2[:, kt, jt * P:(jt + 1) * P],
                                    w3s_sb[:, kt, :],
                                    start=(kt == 0), stop=(kt == DT - 1),
                                    skip_group_check=True)
                                nc.tensor.matmul(
                                    st_ps[:, 1, jt:jt + 1],
                                    sq[:, kt, jt * P:(jt + 1) * P],
                                    ones16[:, kt, 0:1],
                                    start=(kt == 0), stop=(kt == DT - 1),
                                    skip_group_check=True)
                        s1c = work.tile([P, CT], f32, tag="col", bufs=4,
                                        name=f"s1c{uid}")
                        nc.vector.tensor_scalar(
                            out=s1c, in0=st_ps[:, 0, :],
                            scalar1=colc["ccb3"], scalar2=None, op0=ALU.add)
                        sq1 = work.tile([P, CT], f32, tag="col", bufs=4,
                                        name=f"sq1{uid}")
                        nc.vector.tensor_tensor(sq1, s1c, s1c, ALU.mult)
                        U = work.tile([P, CT], f32, tag="col", bufs=4,
                                      name=f"U{uid}")
                        nc.vector.scalar_tensor_tensor(
                            U, st_ps[:, 1, :], colc["cc1"], sq1,
                            ALU.mult, ALU.subtract)
                        return hi, U

                    def stage_a2(c, U, b=b):
                        uid = f"{b}_{c}"
                        lnv = work.tile([P, CT], f32, tag="col", bufs=4,
                                        name=f"lnv{uid}")
                        nc.scalar.activation(lnv, U, AF.Ln, bias=eps_c,
                                             scale=colc["clnv"])
                        a_e = work.tile([P, CT], f32, tag="acol", bufs=3,
                                        name=f"ae{uid}")
                        nc.scalar.activation(a_e, lnv, AF.Exp,
                                             bias=colc["cbe"], scale=-0.5)
                        a_v = work.tile([P, CT], f32, tag="acol", bufs=3,
                                        name=f"av{uid}")
                        nc.vector.tensor_scalar(
                            out=a_v, in0=a_e, scalar1=colc["ccvr"],
                            scalar2=None, op0=ALU.mult)
                        return a_e, a_v

                    def stage_b1(c, hi, a_e, a_v, b=b):
                        if STOP < 2:
                            return c, None, None
                        uid = f"{b}_{c}"
                        h2 = h2b[hi]
                        v_t = work.tile([P, CT, D], fp8, tag="v", bufs=3,
                                        name=f"v{uid}")
                        e_t = work.tile([P, CT, L], fp8, tag="e", bufs=3,
                                        name=f"e{uid}")
                        for jt in range(CT):
                            psv = psum.tile([P, D], f32, tag="work", bufs=4,
                                            name=f"psv{jt}_{uid}")
                            for ks in PAIRS:
                                nc.tensor.matmul(
                                    psv,
                                    h2[:, ks[0]:ks[1], jt * P:(jt + 1) * P],
                                    w3v_sb[:, ks[0]:ks[1], :],
                                    start=(ks[0] == 0), stop=(ks[0] == 2),
                                    perf_mode=DR)
                            nc.scalar.activation(v_t[:, jt, :], psv, AF.Copy,
                                                 scale=a_v[:, jt:jt + 1])
                            pss = psum.tile([P, L], f32, tag="work", bufs=4,
                                            name=f"pss{jt}_{uid}")
                            for ks in PAIRS:
                                nc.tensor.matmul(
                                    pss,
                                    h2[:, ks[0]:ks[1], jt * P:(jt + 1) * P],
                                    w3q_sb[:, ks[0]:ks[1], :],
                                    start=(ks[0] == 0), stop=(ks[0] == 2),
                                    perf_mode=DR)
                            nc.scalar.activation(e_t[:, jt, :], pss, AF.Exp,
                                                 bias=neg1_c,
                                                 scale=a_e[:, jt:jt + 1])
                        return c, v_t, e_t

                    def stage_b2(c, v_t, e_t, b=b, acc=acc):
                        if STOP < 2 or v_t is None:
                            return
                        first, last = (c == 0), (c == nchunks - 1)
                        for pi, ks in enumerate(PAIRS):
                            for mt in range(DT):
                                nc.tensor.matmul(
                                    acc[:, mt, :],
                                    v_t[:, ks[0]:ks[1], mt * P:(mt + 1) * P],
                                    e_t[:, ks[0]:ks[1], :],
                                    start=(first and pi == 0),
                                    stop=(last and pi == 1),
                                    perf_mode=DR, skip_group_check=True)
                            nc.tensor.matmul(
                                acc[0:1, 3, :],
                                ones16[:, ks[0]:ks[1], 0:1],
                                e_t[:, ks[0]:ks[1], :],
                                start=(first and pi == 0),
                                stop=(last and pi == 1),
                                perf_mode=DR, skip_group_check=True)

                    def epi1(b=b, acc=acc):
                        ub = f"b{b}"
                        if STOP < 3:
                            outn = work.tile([P, DT, L], f32r, tag="outn",
                                             bufs=1, name=f"outn{ub}")
                            nc.vector.memset(outn, 0.01)
                            return outn
                        rec = work.tile([1, L], f32r, tag="row", bufs=2,
                                        name=f"rec{ub}")
                        with nc.allow_low_precision("f32r is full fp32"):
                            nc.vector.reciprocal(rec, acc[0:1, 3, :])
                        ps_rb = psum.tile([P, L], f32, tag="work", bufs=4,
                                          name=f"psrb{ub}")
                        nc.tensor.matmul(ps_rb, ones_row, rec,
                                         start=True, stop=True)
                        rb = work.tile([P, L], f32, tag="rb", bufs=1,
                                       name=f"rb{ub}")
                        nc.vector.tensor_copy(rb, ps_rb)
                        outn = work.tile([P, DT, L], f32r, tag="outn", bufs=1,
                                         name=f"outn{ub}")
                        for mt in range(DT):
                            nc.vector.tensor_tensor(outn[:, mt, :],
                                                    acc[:, mt, :], rb,
                                                    ALU.mult)
                        return outn

                    def epi2(outn, b=b):
                        ub = f"b{b}"
                        if STOP < 4:
                            x1n = work.tile([P, LT, D], f32, tag="x1n",
                                            bufs=1, name=f"x1n{ub}")
                            nc.vector.memset(x1n, 0.01)
                            return x1n
                        x1T = work.tile([P, DT, L], f32, tag="x1T", bufs=1,
                                        name=f"x1T{ub}")
                        for mt in range(DT):
                            ps = psum.tile([P, L], f32, tag="work", bufs=4,
                                           name=f"px1T{mt}{ub}")
                            for kt in range(DT):
                                nc.tensor.matmul(
                                    ps, wo_sb[:, kt, mt * P:(mt + 1) * P],
                                    outn[:, kt, :],
                                    start=(kt == 0), stop=(kt == DT - 1))
                            nc.vector.tensor_tensor(x1T[:, mt, :], ps,
                                                    lqbT_sb[:, mt, :], ALU.add)
                        x1n = work.tile([P, LT, D], f32, tag="x1n", bufs=1,
                                        name=f"x1n{ub}")
                        if SUB < 2:
                            nc.vector.memset(x1n, 0.01)
                            nc.vector.memset(fT[:, 0:3, :], 0.01)
                            return x1n
                        for lt in range(LT):
                            ps = psum.tile([P, D], f32, tag="work", bufs=4,
                                           name=f"px1n{lt}{ub}")
                            for kt in range(DT):
                                nc.tensor.matmul(
                                    ps, outn[:, kt, lt * P:(lt + 1) * P],
                                    wo_sb[:, kt, :],
                                    start=(kt == 0), stop=(kt == DT - 1))
                            nc.vector.tensor_tensor(x1n[:, lt, :], ps,
                                                    lqbn_sb[:, lt, :], ALU.add)
                        if SUB < 3:
                            nc.vector.memset(fT[:, 0:3, :], 0.01)
                            return x1n
                        # FF LN stats, per-latent column form from x1n
                        sf1 = work.tile([P, LT], f32, tag="col", bufs=4,
                                        name=f"sf1{ub}")
                        sf2 = work.tile([P, LT], f32, tag="col", bufs=4,
                                        name=f"sf2{ub}")
                        x1sq = work.tile([P, D], f32, tag="x1sq", bufs=2,
                                         name=f"x1sq{ub}")
                        for lt in range(LT):
                            nc.vector.tensor_reduce(
                                sf1[:, lt:lt + 1], x1n[:, lt, :],
                                mybir.AxisListType.X, ALU.add)
                            nc.gpsimd.tensor_tensor(x1sq, x1n[:, lt, :],
                                                    x1n[:, lt, :], ALU.mult)
                            nc.vector.tensor_reduce(
                                sf2[:, lt:lt + 1], x1sq,
                                mybir.AxisListType.X, ALU.add)
                        sqf = work.tile([P, LT], f32, tag="col", bufs=4,
                                        name=f"sqf{ub}")
                        nc.vector.tensor_tensor(sqf, sf1, sf1, ALU.mult)
                        Uf = work.tile([P, LT], f32, tag="col", bufs=4,
                                       name=f"Uf{ub}")
                        nc.vector.scalar_tensor_tensor(
                            Uf, sf2, float(D), sqf, ALU.mult, ALU.subtract)
                        lnvf = work.tile([P, LT], f32, tag="col", bufs=4,
                                         name=f"lnvf{ub}")
                        nc.scalar.activation(lnvf, Uf, AF.Ln, bias=eps_c,
                                             scale=1.0 / (D * D))
                        a_f = work.tile([P, LT], f32r, tag="col", bufs=4,
                                        name=f"af{ub}")
                        nc.scalar.activation(a_f, lnvf, AF.Exp, scale=-0.5)
                        if SUB < 4:
                            nc.vector.memset(fT[:, 0:3, :], 0.01)
                            return x1n
                        # columns -> row -> broadcast -> fT = x1T * a
                        ps_t = psum.tile([1, L], f32, tag="work", bufs=4,
                                         name=f"pst{ub}")
                        for lt in range(LT):
                            nc.tensor.matmul(ps_t[0:1, lt * P:(lt + 1) * P],
                                             a_f[:, lt:lt + 1], ident,
                                             start=True, stop=True,
                                             skip_group_check=True)
                        a_row = work.tile([1, L], f32r, tag="row", bufs=2,
                                          name=f"arow{ub}")
                        nc.vector.tensor_copy(a_row, ps_t)
                        ps_ab = psum.tile([P, L], f32, tag="work", bufs=4,
                                          name=f"psab{ub}")
                        nc.tensor.matmul(ps_ab, ones_row, a_row,
                                         start=True, stop=True)
                        for kt in range(DT):
                            nc.vector.tensor_tensor(fT[:, kt, :],
                                                    x1T[:, kt, :], ps_ab,
                                                    ALU.mult)
                        return x1n

                    def epi3(x1n, b=b):
                        ub = f"b{b}"
                        if STOP < 5:
                            nc.sync.dma_start(
                                y[b].rearrange("(t p) d -> p t d", p=P), x1n)
                            return
                        f2 = work.tile([P, FFT, L], fp8, tag="f2", bufs=1,
                                       name=f"f2{ub}")
                        for mt in range(FFT):
                            ps_a = psum.tile([P, L], f32, tag="work", bufs=4,
                                             name=f"pfa{mt}{ub}")
                            ps_g = psum.tile([P, L], f32, tag="work", bufs=4,
                                             name=f"pfg{mt}{ub}")
                            for ks in PAIRS:
                                nc.tensor.matmul(
                                    ps_a,
                                    fw1_sb[:, ks[0]:ks[1],
                                           mt * P:(mt + 1) * P],
                                    fT[:, ks[0]:ks[1], :],
                                    start=(ks[0] == 0), stop=(ks[0] == 2),
                                    perf_mode=DR)
                            for ks in PAIRS:
                                nc.tensor.matmul(
                                    ps_g,
                                    fw1_sb[:, ks[0]:ks[1],
                                           (FFT + mt) * P:(FFT + mt + 1) * P],
                                    fT[:, ks[0]:ks[1], :],
                                    start=(ks[0] == 0), stop=(ks[0] == 2),
                                    perf_mode=DR)
                            g_sb = work.tile([P, L], bf16, tag="g", bufs=2,
                                             name=f"g{mt}{ub}")
                            nc.scalar.activation(g_sb, ps_g, gelu_af,
                                                 bias=bug_sb[:, mt:mt + 1],
                                                 scale=colc["ccgs"])
                            nc.vector.scalar_tensor_tensor(
                                f2[:, mt, :], ps_a, bua_sb[:, mt:mt + 1],
                                g_sb, ALU.add, ALU.mult)
                        y_sb = work.tile([P, LT, D], f32, tag="y", bufs=1,
                                         name=f"y{ub}")
                        for lt in range(LT):
                            ps = psum.tile([P, D], f32, tag="work", bufs=4,
                                           name=f"py{lt}{ub}")
                            for kk in range(0, FFT, 2):
                                nc.tensor.matmul(
                                    ps,
                                    f2[:, kk:kk + 2, lt * P:(lt + 1) * P],
                                    fw2_sb[:, kk:kk + 2, :],
                                    start=(kk == 0), stop=(kk == FFT - 2),
                                    perf_mode=DR)
                            nc.vector.scalar_tensor_tensor(
                                y_sb[:, lt, :], ps, colc["ccy"],
                                x1n[:, lt, :], ALU.mult, ALU.add)
                        nc.sync.dma_start(
                            y[b].rearrange("(t p) d -> p t d", p=P), y_sb)

                    batches.append((stage_a, stage_a2, stage_b1,
                                    stage_b2, epi1, epi2, epi3))

                # ---- continuous cross-batch software pipeline ----
                b1q, b2q = [], []
                pend_age = 0

                def tick_pending():
                    nonlocal pending, pending_res, pend_age
                    if pending is not None:
                        pend_age += 1
                        if pend_age == 2 and pending_res is None:
                            pending_res = pending[0]()
                        elif pend_age >= 4:
                            flush_pending()

                def pump(drain=False):
                    nonlocal pending, pending_res, pend_age
                    r1 = None
                    if b1q and (drain or len(b1q) > 1):
                        SB1, SB2, E1, E2, E3, c, hi, a_e, a_v, last = \
                            b1q.pop(0)
                        v = SB1(c, hi, a_e, a_v)
                        r1 = (SB2, E1, E2, E3, v, last)
                    if b2q:
                        SB2, E1, E2, E3, v, last = b2q.pop(0)
                        SB2(*v)
                        if last:
                            outn_b = E1()
                            pending = (lambda o=outn_b, e2=E2: e2(o), E3)
                            pending_res = None
                            pend_age = 0
                    if r1 is not None:
                        b2q.append(r1)

                for gc in range(bpc * nchunks):
                    bb, c = divmod(gc, nchunks)
                    SA, SA2, SB1, SB2, E1, E2, E3 = batches[bb]
                    pump()
                    tick_pending()
                    hi, U = SA(c, gc % 3)
                    a_e, a_v = SA2(c, U)
                    b1q.append((SB1, SB2, E1, E2, E3, c, hi, a_e, a_v,
                                c == nchunks - 1))
                while b1q or b2q:
                    pump(drain=True)
                    tick_pending()
                flush_pending()

            if repeat is not None and repeat > 1:
                with tc.For_i(0, repeat, 1):
                    _run()
            else:
                _run()

    nc.compile()
    return nc


def _pow2(x):
    return float(2.0 ** np.round(np.log2(max(float(x), 1e-30))))


def host_prep(inputs, n_points=N_FULL):
    """Fold LN/means/biases into weights, pick fp8 scales, build in_maps."""
    f = lambda a: np.ascontiguousarray(np.asarray(a), dtype=np.float32)
    e4 = ml_dtypes.float8_e4m3fn
    x = f(inputs["x"])[:, :n_points, :]
    query = f(inputs["query"])[0]  # [L, D]

    W0, b0 = f(inputs["mlp_w0"]), f(inputs["mlp_b0"])
    W1, b1 = f(inputs["mlp_w1"]), f(inputs["mlp_b1"])
    W2, b2 = f(inputs["mlp_w2"]), f(inputs["mlp_b2"])
    W3, b3 = f(inputs["mlp_w3"]), f(inputs["mlp_b3"])

    # query path (batch independent, exact)
    gq, bq = f(inputs["ln_q_g"]), f(inputs["ln_q_b"])
    m = query.mean(-1, keepdims=True)
    v = query.var(-1, keepdims=True)
    qn = (query - m) / np.sqrt(v + EPS) * gq + bq
    q = qn @ f(inputs["wq"])  # [L, D]

    gctx, bctx = f(inputs["ln_ctx_g"]), f(inputs["ln_ctx_b"])
    wkv = f(inputs["wkv"])
    gwk = wkv[:, :D] * gctx[:, None]
    gwv = wkv[:, D:] * gctx[:, None]
    bv_const = bctx @ wkv[:, D:]           # beta @ wv

    wq2 = gwk @ q.T                        # [D, L]
    wq2c = wq2 - wq2.mean(0, keepdims=True)
    gwvc = gwv - gwv.mean(0, keepdims=True)
    W3q = W3 @ wq2c                        # [D, L]
    W3v = W3 @ gwvc                        # [D, D]
    bq_row = b3 @ wq2c                     # [L]
    bvv_row = b3 @ gwvc                    # [D]
    W3rs = W3.sum(1)                       # [D]

    # ---- sampled forward for activation-scale selection ----
    xs = x[0, :: max(1, n_points // 2048), :]
    relu = lambda a: np.maximum(a, 0.0)
    h0s = relu(xs @ W0 + b0)
    h1s = relu(h0s @ W1 + b1)
    h2s = relu(h1s @ W2 + b2)
    ctxs = h2s @ W3 + b3
    ms = ctxs.mean(-1, keepdims=True)
    vs = ctxs.var(-1, keepdims=True)
    a_s = 1.0 / np.sqrt(vs + EPS)
    v_s = (ctxs @ gwvc) * a_s              # sampled v minus bias
    logit = (ctxs @ wq2c) * a_s * SCALE    # [ns, L]
    attn = np.exp(logit - logit.max(0, keepdims=True))
    attn = attn / attn.sum(0, keepdims=True)
    out_s = attn.T @ (v_s + bv_const)      # [L, D]
    x1s = out_s @ f(inputs["wo"]) + f(inputs["bo"]) + query
    gff, bff = f(inputs["ln_ff_g"]), f(inputs["ln_ff_b"])
    mf = x1s.mean(-1, keepdims=True)
    vf = x1s.var(-1, keepdims=True)
    fns = (x1s - mf) / np.sqrt(vf + EPS)
    fw1_ = f(inputs["ff_w1"])
    gw1 = fw1_ * gff[:, None]
    bu_full = f(inputs["ff_b1"]) + bff @ fw1_
    gw1c = gw1 - gw1.mean(0, keepdims=True)
    aside_s = fns @ gw1c[:, :FF] + bu_full[:FF]
    gside_s = fns @ gw1c[:, FF:] + bu_full[FF:]
    _erf = np.vectorize(math.erf)
    gel_s = gside_s * 0.5 * (1.0 + _erf(gside_s / math.sqrt(2)))
    f2s = aside_s * gel_s

    sd = lambda a: max(float(np.std(a)), 1e-12)
    s0 = _pow2(1.0 / sd(h0s))
    s1 = _pow2(1.0 / sd(h1s))
    s2 = _pow2(1.0 / sd(h2s))
    ksq = _pow2(1.5 / sd(ctxs))
    k8v = _pow2(1.0 / sd(v_s))

    wsig = 0.25
    kw = lambda Wm: _pow2(wsig / sd(Wm))

    def aug(Wm, brow):
        M = Wm.shape[1] if Wm.ndim == 2 else 1
        out = np.zeros((P * KT, M), np.float32)
        out[:D] = Wm.reshape(D, M)
        out[D] = brow
        return out

    c1b = kw(W1 * (s1 / s0))
    W1p = aug(W1 * (s1 / s0) * c1b, b1 * s1 * c1b)
    c2b = kw(W2 * (s2 / s1))
    W2p = aug(W2 * (s2 / s1) * c2b, b2 * s2 * c2b)
    c3b = kw(W3 * (ksq / s2))
    W3p = aug(W3 * (ksq / s2) * c3b, b3 * ksq * c3b)
    kq = kw(W3q / s2)
    W3qp = aug(W3q * (kq / s2), bq_row * kq)
    kv = kw(W3v / s2)
    W3vp = aug(W3v * (kv / s2), bvv_row * kv)
    km = kw(W3rs)
    w3sp = aug((W3rs * km)[:, None], 0.0)
    ccb3_v = km * s2 * float(b3.sum())  # S1 bias, added in column math

    # stats: S1 = km*s2*D*m; sq = (ksq*ctx)^2 (csq = 1/c3b inside Square)
    kvar = (km * s2 * D) ** 2
    cc1_v = kvar / (D * ksq * ksq)
    clnv_v = 1.0 / kvar
    cbe_v = math.log(SCALE / (kq * s2)) + math.log(s2)  # = log(SCALE/kq)
    cbe_v = math.log(SCALE / kq)
    ccvr_v = (kq * k8v) / (SCALE * kv)

    # FF: fw1a scaled by kfa (weight range, f2 range); gelu side by kg
    kfa = kw(gw1c[:, :FF])
    while kfa * sd(f2s) > 8.0:
        kfa *= 0.5
    while kfa * sd(f2s) < 0.25:
        kfa *= 2.0
    kg = kw(gw1c[:, FF:])
    fw1p = np.zeros((P * KT, FF2), np.float32)
    fw1p[:D, :FF] = gw1c[:, :FF] * kfa
    fw1p[:D, FF:] = gw1c[:, FF:] * kg
    fw2_ = f(inputs["ff_w2"])
    kf2 = kw(fw2_ / kfa)
    fw2p = fw2_ * (kf2 / kfa)

    ccgs_v = 1.0 / kg
    ccy_v = 1.0 / kf2
    bua_v = np.ascontiguousarray((bu_full[:FF] * kfa).reshape(FFT, P).T)
    bug_v = np.ascontiguousarray(bu_full[FF:].reshape(FFT, P).T)

    bo_p = f(inputs["bo"]) + bv_const @ f(inputs["wo"])
    lqbT_v = np.ascontiguousarray(query.T + bo_p[:, None])
    lqbn_v = query + bo_p[None, :] + f(inputs["ff_b2"])[None, :]

    col = lambda val: np.full((P, 1), val, np.float32)
    common = {
        "w0": np.ascontiguousarray(
            np.concatenate([W0 * s0, (b0 * s0)[None, :]], 0)),
        "w1": W1p.astype(e4), "w2": W2p.astype(e4), "w3": W3p.astype(e4),
        "w3q": W3qp.astype(e4), "w3v": W3vp.astype(e4),
        "w3s": w3sp.astype(e4),
        "wo": f(inputs["wo"]) * (1.0 / k8v),
        "fw1": fw1p.astype(e4), "fw2": fw2p.astype(e4),
        "lqbT": lqbT_v, "lqbn": lqbn_v,
        "ident": np.eye(P, dtype=np.float32),
        "onesr": np.ones((1, P), np.float32),
        "ones16": np.concatenate(
            [np.ones((P * KT, 1), np.float32),
             np.zeros((P * KT, 15), np.float32)], 1).astype(e4),
        "padh": np.concatenate(
            [np.ones((1, CHUNK), np.float32),
             np.zeros((P - 1, CHUNK), np.float32)], 0).astype(e4),
        "padz": np.zeros((P, CHUNK), e4),
        "ch1": col(1.0 / c1b), "ch2": col(1.0 / c2b),
        "csq": col(1.0 / c3b),
        "cc1": col(cc1_v), "clnv": col(clnv_v), "cbe": col(cbe_v),
        "ccvr": col(ccvr_v), "ccgs": col(ccgs_v), "ccy": col(ccy_v),
        "ccb3": col(ccb3_v),
        "bua": bua_v, "bug": bug_v,
    }
    in_maps = []
    for c in range(NCORES):
        xs_ = x[c * BPC:(c + 1) * BPC]  # [BPC, n, 3]
        xTs = np.concatenate(
            [xs_.transpose(0, 2, 1),
             np.ones((BPC, 1, n_points), np.float32)], axis=1)
        in_maps.append({"xT": np.ascontiguousarray(xTs), **common})
    return in_maps


_NC_CACHE = {}


def get_nc(n_points=N_FULL):
    if n_points not in _NC_CACHE:
        _NC_CACHE[n_points] = build_nc(n_points)
    return _NC_CACHE[n_points]


def kernel(**inputs):
    from concourse.bass_utils import run_bass_kernel_spmd
    nc = get_nc(N_FULL)
    in_maps = host_prep(inputs, N_FULL)
    res = run_bass_kernel_spmd(nc, in_maps, core_ids=list(range(NCORES)))
    y = np.concatenate([r["y"] for r in res.results], axis=0)
    return y.astype(np.float32)
